# revision 1
# baseline (speedup 1.0000x reference)
"""Trainium2 Bass kernel for nn_GemNetOutput (segment_reduce + FiLM + MLP head).

Reference computation (all fp32):
    g     = segment_sum(x, batch, num_segments=B)        # [B, H]
    gamma = domain_emb @ gamma_w.T + gamma_b             # [B, H]
    beta  = domain_emb @ beta_w.T  + beta_b              # [B, H]
    g     = gamma * g + beta
    h     = silu(g @ w1.T + b1)                          # [B, H]
    h     = silu(h @ w2.T + b2)                          # [B, H/2]
    out   = (h @ w3.T + b3).squeeze(-1)                  # [B]

Shapes: N=1e6 nodes, B=16384 graphs, H=512, FD=16.  `batch` is SORTED.

Strategy (8 NeuronCores, no collectives needed):
  - Shard by SEGMENT range: core c owns segments [c*2048, (c+1)*2048).  Since
    `batch` is sorted, each core's nodes are one contiguous slice of x.
  - Per core, process 16 windows of 128 segments.  For each window the host
    pads the window's node rows to a fixed tile count T (sentinel batch ids
    mask the padding), so the device program is fully static.
  - segment_sum on the PE: for each 128-node tile, build a one-hot
    [node, seg-in-window] matrix on-chip (tensor_scalar is_equal against an
    iota row) and accumulate matmuls into a PSUM [128 seg, 512] tile.
  - x is converted to bf16 on the host (halves the 2 GB HBM read; the
    one-hot matmul accumulates in fp32 so only the input rounding matters).
  - FiLM + MLP run per-window on-device in transposed [feature, seg] layout
    (PE transpose), with biases folded in via a ones-row / per-partition
    activation bias.  Everything after the segment sum is fp32.
"""

import sys
from contextlib import ExitStack

for _p in ("/opt/trn_rl_repo", "/opt/pypackages"):
    if _p not in sys.path:
        sys.path.append(_p)

import ml_dtypes
import numpy as np

import concourse.bass as bass
import concourse.tile as tile
from concourse import bacc, mybir
from concourse import bass_utils

dt = mybir.dt

# Problem constants (hardcoded per the contract).
N_NODES = 1_000_000
B_SEGS = 16_384
H = 512
H2 = 256
FD = 16
N_CORES = 8
SEG_W = 128          # segments per window (PSUM partition dim)

BF16 = ml_dtypes.bfloat16

# x path dtype: "bf16" (half the HBM traffic) or "f32" (exact fallback).
X_MODE = "bf16"

# CoreSim has no Silu LUT; compose silu = z * sigmoid(z) when True (sim tests).
SILU_COMPOSE = False


def _f32_to_bf16_bits(a: np.ndarray) -> np.ndarray:
    """Round-to-nearest-even f32 -> bf16, returned as uint16 bit pattern."""
    u = np.ascontiguousarray(a, dtype=np.float32).view(np.uint32)
    r = (u + np.uint32(0x7FFF) + ((u >> np.uint32(16)) & np.uint32(1))) >> np.uint32(16)
    return r.astype(np.uint16)


def build_program(spc: int, t_tiles: int, xt: int, n_cores: int, x_mode: str = X_MODE):
    """Build the per-core Bass/Tile program.

    spc: segments per core (multiple of 128)
    t_tiles: node tiles (of 128) per 128-segment window, multiple of xt
    xt: node subtiles per x DMA
    """
    windows = spc // SEG_W
    npw = SEG_W * t_tiles          # padded nodes per window
    npad = windows * npw           # padded nodes per core
    x_dt = dt.bfloat16 if x_mode == "bf16" else dt.float32
    m_dt = x_dt                    # MLP matmul dtype (fp32 runs 2 HW passes)

    nc = bacc.Bacc(
        "TRN2",
        target_bir_lowering=False,
        debug=False,
        enable_asserts=False,
        num_devices=n_cores,
    )

    xp = nc.dram_tensor("xp", [npad, H], x_dt, kind="ExternalInput").ap()
    brt = nc.dram_tensor("brt", [windows, 128, t_tiles], dt.float32, kind="ExternalInput").ap()
    dombT = nc.dram_tensor("dombT", [FD + 1, spc], dt.float32, kind="ExternalInput").ap()
    gw = nc.dram_tensor("gw", [FD + 1, H], dt.float32, kind="ExternalInput").ap()
    bw = nc.dram_tensor("bw", [FD + 1, H], dt.float32, kind="ExternalInput").ap()
    w1t = nc.dram_tensor("w1t", [H, H], m_dt, kind="ExternalInput").ap()
    w2t = nc.dram_tensor("w2t", [H, H2], m_dt, kind="ExternalInput").ap()
    w3c = nc.dram_tensor("w3c", [128, H2 // 128], m_dt, kind="ExternalInput").ap()
    b1c = nc.dram_tensor("b1c", [128, H // 128], dt.float32, kind="ExternalInput").ap()
    b2c = nc.dram_tensor("b2c", [128, H2 // 128], dt.float32, kind="ExternalInput").ap()
    b3c = nc.dram_tensor("b3c", [1, 1], dt.float32, kind="ExternalInput").ap()
    iden = nc.dram_tensor("iden", [128, 128], dt.float32, kind="ExternalInput").ap()
    iotr = nc.dram_tensor("iotr", [128, 128], x_dt, kind="ExternalInput").ap()
    out = nc.dram_tensor("out", [1, spc], dt.float32, kind="ExternalOutput").ap()

    HC = H // 128       # 4 h-chunks
    JC = H // 128       # 4 layer-1 output chunks
    KC = H2 // 128      # 2 layer-2 output chunks

    with tile.TileContext(nc) as tc, ExitStack() as ctx:
        cpool = ctx.enter_context(tc.tile_pool(name="consts", bufs=1))
        xpool = ctx.enter_context(tc.tile_pool(name="x", bufs=6))
        bpool = ctx.enter_context(tc.tile_pool(name="brt", bufs=2))
        ohpool = ctx.enter_context(tc.tile_pool(name="oh", bufs=8))
        spool = ctx.enter_context(tc.tile_pool(name="work", bufs=2))
        pg = ctx.enter_context(tc.tile_pool(name="pg", bufs=3, space=bass.MemorySpace.PSUM))
        pt = ctx.enter_context(tc.tile_pool(name="pt", bufs=2, space=bass.MemorySpace.PSUM))
        pm = ctx.enter_context(tc.tile_pool(name="pm", bufs=2, space=bass.MemorySpace.PSUM))

        # ---- constants / weights into SBUF ----
        iden_sb = cpool.tile([128, 128], dt.float32)
        nc.sync.dma_start(iden_sb[:], iden)
        iotr_sb = cpool.tile([128, 128], x_dt)
        nc.sync.dma_start(iotr_sb[:], iotr)
        w1_sb = cpool.tile([128, HC, H], m_dt)
        nc.sync.dma_start(w1_sb[:], w1t.rearrange("(c p) j -> p c j", p=128))
        w2_sb = cpool.tile([128, HC, H2], m_dt)
        nc.sync.dma_start(w2_sb[:], w2t.rearrange("(c p) j -> p c j", p=128))
        w3_sb = cpool.tile([128, KC], m_dt)
        nc.sync.dma_start(w3_sb[:], w3c)
        b1_sb = cpool.tile([128, JC], dt.float32)
        nc.sync.dma_start(b1_sb[:], b1c)
        b2_sb = cpool.tile([128, KC], dt.float32)
        nc.sync.dma_start(b2_sb[:], b2c)
        b3_sb = cpool.tile([1, 1], dt.float32)
        nc.sync.dma_start(b3_sb[:], b3c)
        gw_sb = cpool.tile([FD + 1, H], dt.float32)
        nc.sync.dma_start(gw_sb[:], gw)
        bw_sb = cpool.tile([FD + 1, H], dt.float32)
        nc.sync.dma_start(bw_sb[:], bw)
        domT_sb = cpool.tile([FD + 1, spc], dt.float32)
        nc.sync.dma_start(domT_sb[:], dombT)
        out_sb = cpool.tile([1, spc], dt.float32)

        is_eq = mybir.AluOpType.is_equal

        # ---- PE warm-up: ~5us of dummy matmuls while DMA prefills, so HAM
        # flips to K=8/8 before the real stream starts.
        warm_t = pm.tile([128, H], dt.float32, tag="pmlp")
        for i in range(48):
            nc.tensor.matmul(
                warm_t[:, 0:128], iotr_sb[:], iotr_sb[:],
                start=(i == 0), stop=(i == 47))

        GRP = 4  # windows per gamma/beta matmul group (N = GRP*128 <= 512)
        gbg = {}

        def emit_gamma_beta(wg):
            span = min(GRP * SEG_W, spc - wg * SEG_W)
            g_sbt = spool.tile([128, HC, span], dt.float32, tag="gbg_g")
            b_sbt = spool.tile([128, HC, span], dt.float32, tag="gbg_b")
            dom_s = domT_sb[:, wg * SEG_W: wg * SEG_W + span]
            for hc in range(HC):
                for wsb, dst in ((gw_sb, g_sbt), (bw_sb, b_sbt)):
                    pgb_t = pm.tile([128, H], dt.float32, tag="pmlp")
                    nc.tensor.matmul(
                        pgb_t[:, 0:span],
                        wsb[:, hc * 128:(hc + 1) * 128], dom_s,
                        start=True, stop=True)
                    nc.scalar.copy(dst[:, hc, :], pgb_t[:, 0:span])
            gbg[wg] = (g_sbt, b_sbt)

        for w in range(windows):
            if w % GRP == 0:
                emit_gamma_beta(w)
            # --- batch-relative ids for this window: [128 part, t_tiles] ---
            brt_sb = bpool.tile([128, t_tiles], dt.float32)
            nc.sync.dma_start(brt_sb[:], brt[w])

            # --- segment-sum for this window: accumulate [128 seg, H] ---
            pg_t = pg.tile([128, H], dt.float32)
            base = w * npw
            for blk in range(t_tiles // xt):
                x_sb = xpool.tile([128, xt, H], x_dt)
                rows = xp[base + blk * xt * 128: base + (blk + 1) * xt * 128, :]
                nc.sync.dma_start(x_sb[:], rows.rearrange("(c p) h -> p c h", p=128))
                for c in range(xt):
                    ti = blk * xt + c
                    oh = ohpool.tile([128, 128], x_dt)
                    nc.vector.tensor_scalar(
                        oh[:], iotr_sb[:], brt_sb[:, ti:ti + 1], None, is_eq)
                    nc.tensor.matmul(
                        pg_t[:], oh[:], x_sb[:, c, :],
                        start=(ti == 0), stop=(ti == t_tiles - 1))

            # --- evict g to SBUF, transpose, apply FiLM ---
            g_sb = spool.tile([128, H], dt.float32, tag="g")
            nc.scalar.copy(g_sb[:], pg_t[:])
            pt_t = pt.tile([128, H], dt.float32)
            for hc in range(HC):
                nc.tensor.transpose(
                    pt_t[:, hc * 128:(hc + 1) * 128],
                    g_sb[:, hc * 128:(hc + 1) * 128],
                    iden_sb[:])
            gmodT = spool.tile([128, H], m_dt, tag="gmodT")
            pt_v = pt_t[:].rearrange("p (c s) -> p c s", c=HC)
            gm_v = gmodT[:].rearrange("p (c s) -> p c s", c=HC)
            g_sbt, b_sbt = gbg[(w // GRP) * GRP]
            lo = (w % GRP) * SEG_W
            nc.vector.tensor_mul(gm_v, pt_v, g_sbt[:, :, lo:lo + SEG_W])
            nc.vector.tensor_add(gm_v, gm_v, b_sbt[:, :, lo:lo + SEG_W])

            # --- MLP layer 1: h1T[j, s] = silu(sum_h w1t[h, j] gmodT[h, s] + b1[j]) ---
            ph1 = pm.tile([128, H], dt.float32, tag="pmlp")
            for jc in range(JC):
                for hc in range(HC):
                    nc.tensor.matmul(
                        ph1[:, jc * 128:(jc + 1) * 128],
                        w1_sb[:, hc, jc * 128:(jc + 1) * 128],
                        gmodT[:, hc * 128:(hc + 1) * 128],
                        start=(hc == 0), stop=(hc == HC - 1))
            h1_sb = spool.tile([128, H], m_dt, tag="h1")
            if SILU_COMPOSE:
                z1 = spool.tile([128, H], dt.float32, tag="z1")
                for jc in range(JC):
                    nc.scalar.activation(
                        z1[:, jc * 128:(jc + 1) * 128],
                        ph1[:, jc * 128:(jc + 1) * 128],
                        mybir.ActivationFunctionType.Identity,
                        bias=b1_sb[:, jc:jc + 1])
                nc.scalar.activation(
                    h1_sb[:], z1[:], mybir.ActivationFunctionType.Sigmoid)
                nc.vector.tensor_mul(h1_sb[:], h1_sb[:], z1[:])
            else:
                for jc in range(JC):
                    nc.scalar.activation(
                        h1_sb[:, jc * 128:(jc + 1) * 128],
                        ph1[:, jc * 128:(jc + 1) * 128],
                        mybir.ActivationFunctionType.Silu,
                        bias=b1_sb[:, jc:jc + 1])

            # --- MLP layer 2 ---
            ph2 = pm.tile([128, H2], dt.float32, tag="pmlp")
            for kc in range(KC):
                for hc in range(HC):
                    nc.tensor.matmul(
                        ph2[:, kc * 128:(kc + 1) * 128],
                        w2_sb[:, hc, kc * 128:(kc + 1) * 128],
                        h1_sb[:, hc * 128:(hc + 1) * 128],
                        start=(hc == 0), stop=(hc == HC - 1))
            h2_sb = spool.tile([128, H2], m_dt, tag="h2")
            if SILU_COMPOSE:
                z2 = spool.tile([128, H2], dt.float32, tag="z2")
                for kc in range(KC):
                    nc.scalar.activation(
                        z2[:, kc * 128:(kc + 1) * 128],
                        ph2[:, kc * 128:(kc + 1) * 128],
                        mybir.ActivationFunctionType.Identity,
                        bias=b2_sb[:, kc:kc + 1])
                nc.scalar.activation(
                    h2_sb[:], z2[:], mybir.ActivationFunctionType.Sigmoid)
                nc.vector.tensor_mul(h2_sb[:], h2_sb[:], z2[:])
            else:
                for kc in range(KC):
                    nc.scalar.activation(
                        h2_sb[:, kc * 128:(kc + 1) * 128],
                        ph2[:, kc * 128:(kc + 1) * 128],
                        mybir.ActivationFunctionType.Silu,
                        bias=b2_sb[:, kc:kc + 1])

            # --- output head: out[s] = sum_k w3[k] h2T[k, s] + b3 ---
            po = pm.tile([1, SEG_W], dt.float32, tag="pmlp")
            for kc in range(KC):
                nc.tensor.matmul(
                    po[:], w3_sb[:, kc:kc + 1],
                    h2_sb[:, kc * 128:(kc + 1) * 128],
                    start=(kc == 0), stop=(kc == KC - 1))
            nc.scalar.activation(
                out_sb[0:1, w * SEG_W:(w + 1) * SEG_W], po[:],
                mybir.ActivationFunctionType.Identity,
                bias=b3_sb[0:1, 0:1])

        nc.sync.dma_start(out, out_sb[:])

    nc.compile()
    return nc


def prepare_core_inputs(
    x, batch, domain_emb, gamma_w, gamma_b, beta_w, beta_b,
    w1, b1, w2, b2, w3, b3,
    spc: int, t_tiles: int, n_cores: int, x_mode: str = X_MODE,
):
    """Slice/pad/transpose the full inputs into one in_map per core."""
    windows = spc // SEG_W
    npw = SEG_W * t_tiles
    npad = windows * npw
    n = x.shape[0]

    batch = np.ascontiguousarray(np.asarray(batch).astype(np.int64))
    x = np.asarray(x, dtype=np.float32)

    m_np = BF16 if x_mode == "bf16" else np.float32
    shared = {
        "gw": np.ascontiguousarray(
            np.concatenate([np.asarray(gamma_w, np.float32).T,
                            np.asarray(gamma_b, np.float32)[None]], axis=0)),
        "bw": np.ascontiguousarray(
            np.concatenate([np.asarray(beta_w, np.float32).T,
                            np.asarray(beta_b, np.float32)[None]], axis=0)),
        "w1t": np.ascontiguousarray(np.asarray(w1, np.float32).T.astype(m_np)),
        "w2t": np.ascontiguousarray(np.asarray(w2, np.float32).T.astype(m_np)),
        "w3c": np.ascontiguousarray(
            np.asarray(w3, np.float32).reshape(H2 // 128, 128).T.astype(m_np)),
        "b1c": np.ascontiguousarray(np.asarray(b1, np.float32).reshape(H // 128, 128).T),
        "b2c": np.ascontiguousarray(np.asarray(b2, np.float32).reshape(H2 // 128, 128).T),
        "b3c": np.asarray(b3, np.float32).reshape(1, 1),
        "iden": np.eye(128, dtype=np.float32),
    }
    if x_mode == "bf16":
        shared["iotr"] = np.tile(np.arange(128, dtype=np.float32), (128, 1)).astype(BF16)
        x_np_dt = BF16
    else:
        shared["iotr"] = np.tile(np.arange(128, dtype=np.float32), (128, 1))
        x_np_dt = np.float32

    dom = np.asarray(domain_emb, np.float32)

    in_maps = []
    for core in range(n_cores):
        seg0 = core * spc
        w_starts = np.searchsorted(
            batch, seg0 + SEG_W * np.arange(windows + 1), side="left")
        xp_c = np.zeros((npad, H), dtype=x_np_dt)
        brt_c = np.full((windows, npw), -1.0e9, dtype=np.float32)
        for w in range(windows):
            s, e = int(w_starts[w]), int(w_starts[w + 1])
            cnt = e - s
            if cnt > npw:
                raise ValueError(f"window overflow: {cnt} > {npw}")
            if cnt == 0:
                continue
            if x_mode == "bf16":
                xp_c[w * npw: w * npw + cnt].view(np.uint16)[:] = \
                    _f32_to_bf16_bits(x[s:e])
            else:
                xp_c[w * npw: w * npw + cnt] = x[s:e]
            brt_c[w, :cnt] = (batch[s:e] - (seg0 + w * SEG_W)).astype(np.float32)
        brt_c = np.ascontiguousarray(
            brt_c.reshape(windows, t_tiles, 128).transpose(0, 2, 1))
        dombT_c = np.ascontiguousarray(
            np.concatenate([dom[seg0:seg0 + spc].T,
                            np.ones((1, spc), np.float32)], axis=0))
        in_maps.append({"xp": xp_c, "brt": brt_c, "dombT": dombT_c, **shared})
    return in_maps


def _pick_t_tiles(batch: np.ndarray, spc: int, n_cores: int, xt: int) -> int:
    """Max padded tile count over all 128-segment windows, rounded to xt."""
    edges = np.arange(0, n_cores * spc + 1, SEG_W)
    starts = np.searchsorted(batch, edges, side="left")
    max_cnt = int(np.max(np.diff(starts))) if len(starts) > 1 else 0
    t = max(1, -(-max_cnt // 128))
    return -(-t // xt) * xt


_PROGRAM_CACHE: dict = {}

XT = 8  # node subtiles (of 128 rows) per x DMA

# Set by test harnesses: request an NTFF trace and stash the raw results.
TRACE = False
LAST_RESULT = None


def kernel(**inputs) -> np.ndarray:
    x = np.asarray(inputs["x"], dtype=np.float32)
    batch = np.ascontiguousarray(np.asarray(inputs["batch"]).astype(np.int64))
    assert x.shape == (N_NODES, H), x.shape

    spc = B_SEGS // N_CORES
    t_tiles = _pick_t_tiles(batch, spc, N_CORES, XT)

    key = (spc, t_tiles, XT, N_CORES, X_MODE)
    if key not in _PROGRAM_CACHE:
        _PROGRAM_CACHE[key] = build_program(spc, t_tiles, XT, N_CORES, X_MODE)
    nc = _PROGRAM_CACHE[key]

    in_maps = prepare_core_inputs(
        x, batch,
        inputs["domain_emb"], inputs["gamma_w"], inputs["gamma_b"],
        inputs["beta_w"], inputs["beta_b"],
        inputs["w1"], inputs["b1"], inputs["w2"], inputs["b2"],
        inputs["w3"], inputs["b3"],
        spc, t_tiles, N_CORES, X_MODE,
    )

    res = bass_utils.run_bass_kernel_spmd(
        nc, in_maps, core_ids=list(range(N_CORES)), trace=TRACE)
    global LAST_RESULT
    LAST_RESULT = res
    out = np.concatenate([res.results[c]["out"].reshape(-1) for c in range(N_CORES)])
    return np.ascontiguousarray(out.astype(np.float32))



# revision 7
# speedup vs baseline: 2.2826x; 2.2826x over previous
"""Trainium2 Bass kernel for nn_GemNetOutput (segment_reduce + FiLM + MLP head).

Reference computation (all fp32):
    g     = segment_sum(x, batch, num_segments=B)        # [B, H]
    gamma = domain_emb @ gamma_w.T + gamma_b             # [B, H]
    beta  = domain_emb @ beta_w.T  + beta_b              # [B, H]
    g     = gamma * g + beta
    h     = silu(g @ w1.T + b1)                          # [B, H]
    h     = silu(h @ w2.T + b2)                          # [B, H/2]
    out   = (h @ w3.T + b3).squeeze(-1)                  # [B]

Shapes: N=1e6 nodes, B=16384 graphs, H=512, FD=16.  `batch` is SORTED.

Strategy (8 NeuronCores, no collectives needed):
  - Shard by SEGMENT range: core c owns segments [c*2048, (c+1)*2048).  Since
    `batch` is sorted, each core's nodes are one contiguous slice of x.
  - Per core, 16 windows of 128 segments; each window's node rows are padded
    to a fixed tile count (sentinel batch ids mask the padding), so the
    device program is fully static.
  - x is converted to fp8-e4m3 on the host with SIGMA-DELTA (error-feedback)
    rounding along each segment: quantization errors telescope within a
    segment, so the device's segment sums match the fp32 sums to ~1 quantum
    instead of sqrt(n) quanta.  Halves HBM traffic vs bf16.
  - x is packed on the host into [block, 128, xt*H] so each DMA is a fully
    contiguous 1 MB transfer (8 KB per partition line).
  - segment_sum on the PE with fp8 DoubleRow matmuls: each matmul consumes
    TWO 128-node tiles (contraction 256) against a [128, 2, 128] one-hot.
  - One-hot built on DVE with one batched tensor_tensor per DMA block
    (stride-0 broadcast APs) instead of one tensor_scalar per tile.
  - beta (incl. beta_b) is folded into the MLP-1 accumulation on the host:
    ph1 += (W1 @ beta_w_ext.T) @ dom_ext, one small K=17 matmul per j-chunk.
  - FiLM multiply + MLP run per GROUP of 4 windows in transposed
    [feature, seg] layout so MLP matmuls have N=512 moving operands.
"""

import sys
from contextlib import ExitStack

for _p in ("/opt/trn_rl_repo", "/opt/pypackages"):
    if _p not in sys.path:
        sys.path.append(_p)

import ml_dtypes
import numpy as np

import concourse.bass as bass
import concourse.tile as tile
from concourse import bacc, mybir
from concourse import bass_utils

dt = mybir.dt

# Problem constants (hardcoded per the contract).
N_NODES = 1_000_000
B_SEGS = 16_384
H = 512
H2 = 256
FD = 16
N_CORES = 8
SEG_W = 128          # segments per window (PSUM partition dim)
GRP = 4              # windows per MLP group (moving N = GRP*SEG_W = 512)

BF16 = ml_dtypes.bfloat16
F8 = ml_dtypes.float8_e4m3fn

# CoreSim has no Silu LUT; compose silu = z * sigmoid(z) when True (sim tests).
SILU_COMPOSE = False


def _f32_to_bf16(a: np.ndarray) -> np.ndarray:
    return np.ascontiguousarray(a, dtype=np.float32).astype(BF16)


def build_program(spc: int, t_tiles: int, xt: int, n_cores: int):
    """Build the per-core Bass/Tile program.

    spc: segments per core (multiple of 128)
    t_tiles: node tiles (of 128) per 128-segment window; even, multiple of xt
    xt: node subtiles per x DMA block; even
    """
    assert t_tiles % 2 == 0 and xt % 2 == 0 and t_tiles % xt == 0
    windows = spc // SEG_W
    nblk = t_tiles // xt           # x DMA blocks per window
    x_dt = dt.float8e4
    m_dt = dt.bfloat16             # MLP matmul dtype
    f32r = dt.float32r             # 1-pass fp32 for gamma/beta-fold matmuls

    nc = bacc.Bacc(
        "TRN2",
        target_bir_lowering=False,
        debug=False,
        enable_asserts=False,
        num_devices=n_cores,
    )

    xp = nc.dram_tensor("xp", [windows * nblk, 128, xt * H], x_dt,
                        kind="ExternalInput").ap()
    brt = nc.dram_tensor("brt", [128, windows * t_tiles], dt.bfloat16,
                         kind="ExternalInput").ap()
    domE = nc.dram_tensor("domE", [FD + 1, spc], f32r, kind="ExternalInput").ap()
    gw = nc.dram_tensor("gw", [FD + 1, H], f32r, kind="ExternalInput").ap()
    w1bw = nc.dram_tensor("w1bw", [FD + 1, H], f32r, kind="ExternalInput").ap()
    w1t = nc.dram_tensor("w1t", [H, H], m_dt, kind="ExternalInput").ap()
    w2t = nc.dram_tensor("w2t", [H, H2], m_dt, kind="ExternalInput").ap()
    w3c = nc.dram_tensor("w3c", [128, H2 // 128], m_dt, kind="ExternalInput").ap()
    b1c = nc.dram_tensor("b1c", [128, H // 128], dt.float32, kind="ExternalInput").ap()
    b2c = nc.dram_tensor("b2c", [128, H2 // 128], dt.float32, kind="ExternalInput").ap()
    b3c = nc.dram_tensor("b3c", [1, 1], dt.float32, kind="ExternalInput").ap()
    iden = nc.dram_tensor("iden", [128, 128], dt.float32, kind="ExternalInput").ap()
    iotr = nc.dram_tensor("iotr", [128, 128], dt.bfloat16, kind="ExternalInput").ap()
    out = nc.dram_tensor("out", [1, spc], dt.float32, kind="ExternalOutput").ap()

    HC = H // 128       # 4 h-chunks
    JC = H // 128       # 4 layer-1 output chunks
    KC = H2 // 128      # 2 layer-2 output chunks
    NG = GRP * SEG_W    # moving width of group-level MLP matmuls

    is_eq = mybir.AluOpType.is_equal
    DR = mybir.MatmulPerfMode.DoubleRow

    with tile.TileContext(nc) as tc, ExitStack() as ctx:
        cpool = ctx.enter_context(tc.tile_pool(name="consts", bufs=1))
        xpool = ctx.enter_context(tc.tile_pool(name="x", bufs=6))
        ohpool = ctx.enter_context(tc.tile_pool(name="oh", bufs=4))
        spool = ctx.enter_context(tc.tile_pool(name="work", bufs=2))
        pg = ctx.enter_context(tc.tile_pool(name="pg", bufs=2, space=bass.MemorySpace.PSUM))
        pt = ctx.enter_context(tc.tile_pool(name="pt", bufs=2, space=bass.MemorySpace.PSUM))
        pm = ctx.enter_context(tc.tile_pool(name="pm", bufs=3, space=bass.MemorySpace.PSUM))

        # ---- constants / weights into SBUF ----
        iden_sb = cpool.tile([128, 128], dt.float32)
        nc.sync.dma_start(iden_sb[:], iden)
        iotr_sb = cpool.tile([128, 128], dt.bfloat16)
        nc.sync.dma_start(iotr_sb[:], iotr)
        w1_sb = cpool.tile([128, HC, H], m_dt)
        nc.sync.dma_start(w1_sb[:], w1t.rearrange("(c p) j -> p c j", p=128))
        w2_sb = cpool.tile([128, HC, H2], m_dt)
        nc.sync.dma_start(w2_sb[:], w2t.rearrange("(c p) j -> p c j", p=128))
        w3_sb = cpool.tile([128, KC], m_dt)
        nc.sync.dma_start(w3_sb[:], w3c)
        b1_sb = cpool.tile([128, JC], dt.float32)
        nc.sync.dma_start(b1_sb[:], b1c)
        b2_sb = cpool.tile([128, KC], dt.float32)
        nc.sync.dma_start(b2_sb[:], b2c)
        b3_sb = cpool.tile([1, 1], dt.float32)
        nc.sync.dma_start(b3_sb[:], b3c)
        gw_sb = cpool.tile([FD + 1, H], f32r)
        nc.sync.dma_start(gw_sb[:], gw)
        w1bw_sb = cpool.tile([FD + 1, H], f32r)
        nc.sync.dma_start(w1bw_sb[:], w1bw)
        domE_sb = cpool.tile([FD + 1, spc], f32r)
        nc.sync.dma_start(domE_sb[:], domE)
        brt_sb = cpool.tile([128, windows * t_tiles], dt.bfloat16)
        nc.sync.dma_start(brt_sb[:], brt)
        out_sb = cpool.tile([1, spc], dt.float32)

        # ---- PE warm-up: ~5us of dummy matmuls while DMA prefills, so HAM
        # flips to K=8/8 before the real stream starts.
        warm_t = pm.tile([128, H], dt.float32, tag="pmlp")
        for i in range(48):
            nc.tensor.matmul(
                warm_t[:, 0:128], iotr_sb[:], iotr_sb[:],
                start=(i == 0), stop=(i == 47))

        gstate = {}

        def emit_gamma(wg, span):
            """gammaT for windows [wg, wg+GRP): [128 h, HC, span] bf16."""
            gam = spool.tile([128, HC, NG], m_dt, tag="gam")
            dom_s = domE_sb[:, wg * SEG_W: wg * SEG_W + span]
            for hc in range(HC):
                pgb = pm.tile([128, H], dt.float32, tag="pmlp")
                nc.tensor.matmul(
                    pgb[:, 0:span],
                    gw_sb[:, hc * 128:(hc + 1) * 128], dom_s,
                    start=True, stop=True)
                nc.scalar.copy(gam[:, hc, 0:span], pgb[:, 0:span])
            gstate["gam"] = gam
            gstate["gmodT"] = spool.tile(
                [128, HC, NG], m_dt, tag="gmodT", name="gmodT")

        def emit_mlp(wg, span):
            """MLP for windows [wg, wg+GRP) from gstate['gmodT']."""
            gmodT = gstate["gmodT"]
            dom_s = domE_sb[:, wg * SEG_W: wg * SEG_W + span]
            # layer 1 (+ folded beta/beta_b via K=17 matmul)
            h1 = spool.tile([128, HC, NG], m_dt, tag="h1")
            for jc in range(JC):
                ph1 = pm.tile([128, NG], dt.float32, tag="pmlp")
                nc.tensor.matmul(
                    ph1[:, 0:span],
                    w1bw_sb[:, jc * 128:(jc + 1) * 128], dom_s,
                    start=True, stop=False, skip_group_check=True)
                for hc in range(HC):
                    nc.tensor.matmul(
                        ph1[:, 0:span],
                        w1_sb[:, hc, jc * 128:(jc + 1) * 128],
                        gmodT[:, hc, 0:span],
                        start=False, stop=(hc == HC - 1), skip_group_check=True)
                if SILU_COMPOSE:
                    z1 = spool.tile([128, NG], dt.float32, tag="z1")
                    nc.scalar.activation(
                        z1[:, 0:span], ph1[:, 0:span],
                        mybir.ActivationFunctionType.Identity,
                        bias=b1_sb[:, jc:jc + 1])
                    nc.scalar.activation(
                        h1[:, jc, 0:span], z1[:, 0:span],
                        mybir.ActivationFunctionType.Sigmoid)
                    nc.vector.tensor_mul(
                        h1[:, jc, 0:span], h1[:, jc, 0:span], z1[:, 0:span])
                else:
                    nc.scalar.activation(
                        h1[:, jc, 0:span], ph1[:, 0:span],
                        mybir.ActivationFunctionType.Silu,
                        bias=b1_sb[:, jc:jc + 1])
            # layer 2
            h2 = spool.tile([128, KC, NG], m_dt, tag="h2")
            for kc in range(KC):
                ph2 = pm.tile([128, NG], dt.float32, tag="pmlp")
                for hc in range(HC):
                    nc.tensor.matmul(
                        ph2[:, 0:span],
                        w2_sb[:, hc, kc * 128:(kc + 1) * 128],
                        h1[:, hc, 0:span],
                        start=(hc == 0), stop=(hc == HC - 1))
                if SILU_COMPOSE:
                    z2 = spool.tile([128, NG], dt.float32, tag="z2")
                    nc.scalar.activation(
                        z2[:, 0:span], ph2[:, 0:span],
                        mybir.ActivationFunctionType.Identity,
                        bias=b2_sb[:, kc:kc + 1])
                    nc.scalar.activation(
                        h2[:, kc, 0:span], z2[:, 0:span],
                        mybir.ActivationFunctionType.Sigmoid)
                    nc.vector.tensor_mul(
                        h2[:, kc, 0:span], h2[:, kc, 0:span], z2[:, 0:span])
                else:
                    nc.scalar.activation(
                        h2[:, kc, 0:span], ph2[:, 0:span],
                        mybir.ActivationFunctionType.Silu,
                        bias=b2_sb[:, kc:kc + 1])
            # output head
            po = pm.tile([1, NG], dt.float32, tag="pmlp")
            for kc in range(KC):
                nc.tensor.matmul(
                    po[:, 0:span], w3_sb[:, kc:kc + 1], h2[:, kc, 0:span],
                    start=(kc == 0), stop=(kc == KC - 1))
            nc.scalar.activation(
                out_sb[0:1, wg * SEG_W: wg * SEG_W + span], po[:, 0:span],
                mybir.ActivationFunctionType.Identity,
                bias=b3_sb[0:1, 0:1])

        for w in range(windows):
            if w % GRP == 0:
                emit_gamma(w, min(NG, spc - w * SEG_W))

            # --- segment-sum for this window: accumulate [128 seg, H] ---
            pg_t = pg.tile([128, H], dt.float32)
            for blk in range(nblk):
                x_sb = xpool.tile([128, xt, H], x_dt)
                nc.sync.dma_start(x_sb[:], xp[w * nblk + blk])
                # batched one-hot for the whole block: [128, xt, 128] fp8
                oh = ohpool.tile([128, xt, 128], x_dt)
                iotr_v = iotr_sb[:].rearrange("p (o s) -> p o s", o=1)
                brt_v = brt_sb[:, w * t_tiles + blk * xt:
                               w * t_tiles + (blk + 1) * xt]
                brt_v = brt_v.rearrange("p (c o) -> p c o", o=1)
                in0, in1 = bass.broadcast_tensor_aps(iotr_v, brt_v)
                nc.vector.tensor_tensor(oh[:], in0, in1, is_eq)
                for gpair in range(xt // 2):
                    ti = blk * xt + 2 * gpair
                    nc.tensor.matmul(
                        pg_t[:],
                        oh[:, 2 * gpair:2 * gpair + 2, :],
                        x_sb[:, 2 * gpair:2 * gpair + 2, :],
                        start=(ti == 0), stop=(ti == t_tiles - 2),
                        perf_mode=DR)

            # --- evict g, transpose, FiLM multiply into group gmodT ---
            g_sb = spool.tile([128, H], dt.float32, tag="g")
            nc.scalar.copy(g_sb[:], pg_t[:])
            pt_t = pt.tile([128, H], dt.float32)
            for hc in range(HC):
                nc.tensor.transpose(
                    pt_t[:, hc * 128:(hc + 1) * 128],
                    g_sb[:, hc * 128:(hc + 1) * 128],
                    iden_sb[:])
            wi = w % GRP
            gam = gstate["gam"]
            gmodT = gstate["gmodT"]
            pt_v = pt_t[:].rearrange("p (c s) -> p c s", c=HC)
            gm_v = gmodT[:].rearrange("p c (g s) -> p c g s", g=GRP)
            ga_v = gam[:].rearrange("p c (g s) -> p c g s", g=GRP)
            nc.vector.tensor_mul(
                gm_v[:, :, wi, :], pt_v, ga_v[:, :, wi, :])

            if w % GRP == GRP - 1 or w == windows - 1:
                wg = (w // GRP) * GRP
                emit_mlp(wg, min(NG, spc - wg * SEG_W))

        nc.sync.dma_start(out, out_sb[:])

    nc.compile()
    return nc


def _sigma_delta_fp8(x: np.ndarray, batch: np.ndarray) -> np.ndarray:
    """fp8-e4m3 quantization of x with per-(segment, h) error feedback.

    Within each segment the quantization errors telescope, so segment sums
    of the returned array match the fp32 sums to ~1 quantum.
    """
    starts = np.searchsorted(batch, np.arange(B_SEGS + 1))
    lens = np.diff(starts)
    L = int(lens.max())
    xq = np.empty(x.shape, dtype=F8)
    order = np.argsort(-lens, kind="stable")  # longest first: shrinking actives
    sorted_lens = lens[order]
    sorted_starts = starts[order]
    carry = np.zeros((B_SEGS, x.shape[1]), np.float32)
    for k in range(L):
        n_act = int(np.searchsorted(-sorted_lens, -k, side="left"))
        if n_act == 0:
            break
        rows = sorted_starts[:n_act] + k
        v = x[rows] + carry[:n_act]
        q = v.astype(F8)
        carry[:n_act] = v - q.astype(np.float32)
        xq[rows] = q
    return xq


def prepare_core_inputs(
    x, batch, domain_emb, gamma_w, gamma_b, beta_w, beta_b,
    w1, b1, w2, b2, w3, b3,
    spc: int, t_tiles: int, xt: int, n_cores: int,
):
    """Slice/pad/pack the full inputs into one in_map per core."""
    windows = spc // SEG_W
    nblk = t_tiles // xt
    npw = SEG_W * t_tiles

    batch = np.ascontiguousarray(np.asarray(batch).astype(np.int64))
    x = np.asarray(x, dtype=np.float32)

    w1_f = np.asarray(w1, np.float32)
    bw_ext = np.concatenate([np.asarray(beta_w, np.float32).T,
                             np.asarray(beta_b, np.float32)[None]], axis=0)  # [17, H]
    w1bw = np.ascontiguousarray(bw_ext @ w1_f.T)                             # [17, H]

    shared = {
        "gw": np.ascontiguousarray(
            np.concatenate([np.asarray(gamma_w, np.float32).T,
                            np.asarray(gamma_b, np.float32)[None]], axis=0)),
        "w1bw": w1bw,
        "w1t": np.ascontiguousarray(_f32_to_bf16(w1_f.T)),
        "w2t": np.ascontiguousarray(_f32_to_bf16(np.asarray(w2, np.float32).T)),
        "w3c": np.ascontiguousarray(
            _f32_to_bf16(np.asarray(w3, np.float32).reshape(H2 // 128, 128).T)),
        "b1c": np.ascontiguousarray(np.asarray(b1, np.float32).reshape(H // 128, 128).T),
        "b2c": np.ascontiguousarray(np.asarray(b2, np.float32).reshape(H2 // 128, 128).T),
        "b3c": np.asarray(b3, np.float32).reshape(1, 1),
        "iden": np.eye(128, dtype=np.float32),
        "iotr": np.tile(np.arange(128, dtype=np.float32), (128, 1)).astype(BF16),
    }

    xq = _sigma_delta_fp8(x, batch)
    xq_u8 = xq.view(np.uint8)

    dom = np.asarray(domain_emb, np.float32)

    in_maps = []
    for core in range(n_cores):
        seg0 = core * spc
        w_starts = np.searchsorted(
            batch, seg0 + SEG_W * np.arange(windows + 1), side="left")
        xp_c = np.zeros((windows, npw, H), dtype=np.uint8)
        brt_c = np.full((windows, npw), -1.0e9, dtype=np.float32)
        for w in range(windows):
            s, e = int(w_starts[w]), int(w_starts[w + 1])
            cnt = e - s
            if cnt > npw:
                raise ValueError(f"window overflow: {cnt} > {npw}")
            if cnt == 0:
                continue
            xp_c[w, :cnt] = xq_u8[s:e]
            brt_c[w, :cnt] = (batch[s:e] - (seg0 + w * SEG_W)).astype(np.float32)
        # [windows, npw, H] -> [windows*nblk, 128, xt*H], node (b*xt+c)*128+p
        xp_c = np.ascontiguousarray(
            xp_c.reshape(windows * nblk, xt, 128, H)
            .transpose(0, 2, 1, 3)
            .reshape(windows * nblk, 128, xt * H)).view(F8)
        # [windows, npw] -> [128, windows*t_tiles]: brt[p, w*t_tiles+ti]
        brt_c = np.ascontiguousarray(
            brt_c.reshape(windows, t_tiles, 128).transpose(2, 0, 1)
            .reshape(128, windows * t_tiles).astype(BF16))
        domE_c = np.ascontiguousarray(
            np.concatenate([dom[seg0:seg0 + spc].T,
                            np.ones((1, spc), np.float32)], axis=0))
        in_maps.append({"xp": xp_c, "brt": brt_c, "domE": domE_c, **shared})
    return in_maps


def _pick_tiling(batch: np.ndarray, spc: int, n_cores: int) -> tuple[int, int]:
    """(t_tiles, xt): max padded tile count (even) and DMA subtile count."""
    edges = np.arange(0, n_cores * spc + 1, SEG_W)
    starts = np.searchsorted(batch, edges, side="left")
    max_cnt = int(np.max(np.diff(starts))) if len(starts) > 1 else 1
    t = max(2, 2 * (-(-max_cnt // 256)))         # even tile count
    # xt: even divisor of t, as close to 16 as possible (1 MB DMA blocks)
    divs = [d for d in range(2, t + 1, 2) if t % d == 0]
    xt = min(divs, key=lambda d: (abs(d - 16), d))
    return t, xt


_PROGRAM_CACHE: dict = {}

# Set by test harnesses: request an NTFF trace and stash the raw results.
TRACE = False
LAST_RESULT = None


def kernel(**inputs) -> np.ndarray:
    x = np.asarray(inputs["x"], dtype=np.float32)
    batch = np.ascontiguousarray(np.asarray(inputs["batch"]).astype(np.int64))
    assert x.shape == (N_NODES, H), x.shape

    spc = B_SEGS // N_CORES
    t_tiles, xt = _pick_tiling(batch, spc, N_CORES)

    key = (spc, t_tiles, xt, N_CORES)
    if key not in _PROGRAM_CACHE:
        _PROGRAM_CACHE[key] = build_program(spc, t_tiles, xt, N_CORES)
    nc = _PROGRAM_CACHE[key]

    in_maps = prepare_core_inputs(
        x, batch,
        inputs["domain_emb"], inputs["gamma_w"], inputs["gamma_b"],
        inputs["beta_w"], inputs["beta_b"],
        inputs["w1"], inputs["b1"], inputs["w2"], inputs["b2"],
        inputs["w3"], inputs["b3"],
        spc, t_tiles, xt, N_CORES,
    )

    res = bass_utils.run_bass_kernel_spmd(
        nc, in_maps, core_ids=list(range(N_CORES)), trace=TRACE)
    global LAST_RESULT
    LAST_RESULT = res
    out = np.concatenate([res.results[c]["out"].reshape(-1) for c in range(N_CORES)])
    return np.ascontiguousarray(out.astype(np.float32))


# revision 9
# speedup vs baseline: 2.3832x; 1.0441x over previous
"""Trainium2 Bass kernel for nn_GemNetOutput (segment_reduce + FiLM + MLP head).

Reference computation (all fp32):
    g     = segment_sum(x, batch, num_segments=B)        # [B, H]
    gamma = domain_emb @ gamma_w.T + gamma_b             # [B, H]
    beta  = domain_emb @ beta_w.T  + beta_b              # [B, H]
    g     = gamma * g + beta
    h     = silu(g @ w1.T + b1)                          # [B, H]
    h     = silu(h @ w2.T + b2)                          # [B, H/2]
    out   = (h @ w3.T + b3).squeeze(-1)                  # [B]

Shapes: N=1e6 nodes, B=16384 graphs, H=512, FD=16.  `batch` is SORTED.

Strategy (8 NeuronCores, no collectives needed):
  - The 16384 segments are BIN-PACKED into 128 windows of exactly 128
    segments each, equalizing the node count per window (greedy LPT on the
    host).  Core c owns windows [c*16, (c+1)*16).  All cores run one
    identical static program; every window is padded to the same t_tiles
    node tiles (sentinel one-hot ids mask the padding) -- balancing makes
    that padding ~1.5% instead of ~5%.
  - x is converted to fp8-e4m3 on the host with SIGMA-DELTA (error-feedback)
    rounding along each segment: quantization errors telescope within a
    segment, so the device's segment sums match the fp32 sums to ~1 quantum
    instead of sqrt(n) quanta.  Halves HBM traffic vs bf16.
  - x is packed on the host into [window, 128, t_tiles*H] so each DMA block
    is a fully contiguous ~1 MB transfer (8 KB per partition line).
  - segment_sum on the PE with fp8 DoubleRow matmuls: each matmul consumes
    TWO 128-node tiles (contraction 256) against a [128, 2, 128] one-hot.
  - One-hot built on DVE with one batched tensor_tensor per DMA block
    (stride-0 broadcast APs) instead of one tensor_scalar per tile.
  - beta (incl. beta_b) is folded into the MLP-1 accumulation on the host:
    ph1 += (W1 @ beta_w_ext.T) @ dom_ext, one small K=17 matmul per j-chunk.
  - FiLM multiply + MLP run per GROUP of 4 windows in transposed
    [feature, seg] layout so MLP matmuls have N=512 moving operands.
"""

import sys
from contextlib import ExitStack

for _p in ("/opt/trn_rl_repo", "/opt/pypackages"):
    if _p not in sys.path:
        sys.path.append(_p)

import ml_dtypes
import numpy as np

import concourse.bass as bass
import concourse.tile as tile
from concourse import bacc, mybir
from concourse import bass_utils

dt = mybir.dt

# Problem constants (hardcoded per the contract).
N_NODES = 1_000_000
B_SEGS = 16_384
H = 512
H2 = 256
FD = 16
N_CORES = 8
SEG_W = 128          # segments per window (PSUM partition dim)
GRP = 4              # windows per MLP group (moving N = GRP*SEG_W = 512)
XT = 16              # max node subtiles (of 128 rows) per x DMA block

BF16 = ml_dtypes.bfloat16
F8 = ml_dtypes.float8_e4m3fn

# CoreSim has no Silu LUT; compose silu = z * sigmoid(z) when True (sim tests).
SILU_COMPOSE = False


def _f32_to_bf16(a: np.ndarray) -> np.ndarray:
    return np.ascontiguousarray(a, dtype=np.float32).astype(BF16)


def _blocks_of(t_tiles: int) -> tuple:
    """Split t_tiles into even-sized DMA blocks of at most XT subtiles."""
    blocks = [XT] * (t_tiles // XT)
    rem = t_tiles % XT
    if rem:
        blocks.append(rem)
    assert all(b % 2 == 0 for b in blocks)
    return tuple(blocks)


def build_program(spc: int, t_tiles: int, n_cores: int):
    """Build the per-core Bass/Tile program.

    spc: segments per core (multiple of 128)
    t_tiles: node tiles (of 128) per 128-segment window; even
    """
    assert t_tiles % 2 == 0
    windows = spc // SEG_W
    blocks = _blocks_of(t_tiles)
    x_dt = dt.float8e4
    m_dt = dt.bfloat16             # MLP matmul dtype

    nc = bacc.Bacc(
        "TRN2",
        target_bir_lowering=False,
        debug=False,
        enable_asserts=False,
        num_devices=n_cores,
    )

    xp = nc.dram_tensor("xp", [windows, 128, t_tiles * H], x_dt,
                        kind="ExternalInput").ap()
    brt = nc.dram_tensor("brt", [128, windows * t_tiles], dt.bfloat16,
                         kind="ExternalInput").ap()
    domE = nc.dram_tensor("domE", [FD + 1, spc], m_dt, kind="ExternalInput").ap()
    gw = nc.dram_tensor("gw", [FD + 1, H], m_dt, kind="ExternalInput").ap()
    w1bw = nc.dram_tensor("w1bw", [FD + 1, H], m_dt, kind="ExternalInput").ap()
    w1t = nc.dram_tensor("w1t", [H, H], m_dt, kind="ExternalInput").ap()
    w2t = nc.dram_tensor("w2t", [H, H2], m_dt, kind="ExternalInput").ap()
    w3c = nc.dram_tensor("w3c", [128, H2 // 128], m_dt, kind="ExternalInput").ap()
    b1c = nc.dram_tensor("b1c", [128, H // 128], dt.float32, kind="ExternalInput").ap()
    b2c = nc.dram_tensor("b2c", [128, H2 // 128], dt.float32, kind="ExternalInput").ap()
    b3c = nc.dram_tensor("b3c", [1, 1], dt.float32, kind="ExternalInput").ap()
    iden = nc.dram_tensor("iden", [128, 128], dt.float32, kind="ExternalInput").ap()
    iotr = nc.dram_tensor("iotr", [128, 128], dt.bfloat16, kind="ExternalInput").ap()
    out = nc.dram_tensor("out", [1, spc], dt.float32, kind="ExternalOutput").ap()

    HC = H // 128       # 4 h-chunks
    JC = H // 128       # 4 layer-1 output chunks
    KC = H2 // 128      # 2 layer-2 output chunks
    NG = GRP * SEG_W    # moving width of group-level MLP matmuls

    is_eq = mybir.AluOpType.is_equal
    DR = mybir.MatmulPerfMode.DoubleRow

    with tile.TileContext(nc) as tc, ExitStack() as ctx:
        cpool = ctx.enter_context(tc.tile_pool(name="consts", bufs=1))
        xpool = ctx.enter_context(tc.tile_pool(name="x", bufs=8))
        ohpool = ctx.enter_context(tc.tile_pool(name="oh", bufs=6))
        spool = ctx.enter_context(tc.tile_pool(name="work", bufs=2))
        pg = ctx.enter_context(tc.tile_pool(name="pg", bufs=2, space=bass.MemorySpace.PSUM))
        pt = ctx.enter_context(tc.tile_pool(name="pt", bufs=2, space=bass.MemorySpace.PSUM))
        pm = ctx.enter_context(tc.tile_pool(name="pm", bufs=3, space=bass.MemorySpace.PSUM))

        # ---- constants / weights into SBUF ----
        iden_sb = cpool.tile([128, 128], dt.float32)
        nc.sync.dma_start(iden_sb[:], iden)
        iotr_sb = cpool.tile([128, 128], dt.bfloat16)
        nc.sync.dma_start(iotr_sb[:], iotr)
        w1_sb = cpool.tile([128, HC, H], m_dt)
        nc.sync.dma_start(w1_sb[:], w1t.rearrange("(c p) j -> p c j", p=128))
        w2_sb = cpool.tile([128, HC, H2], m_dt)
        nc.sync.dma_start(w2_sb[:], w2t.rearrange("(c p) j -> p c j", p=128))
        w3_sb = cpool.tile([128, KC], m_dt)
        nc.sync.dma_start(w3_sb[:], w3c)
        b1_sb = cpool.tile([128, JC], dt.float32)
        nc.sync.dma_start(b1_sb[:], b1c)
        b2_sb = cpool.tile([128, KC], dt.float32)
        nc.sync.dma_start(b2_sb[:], b2c)
        b3_sb = cpool.tile([1, 1], dt.float32)
        nc.sync.dma_start(b3_sb[:], b3c)
        gw_sb = cpool.tile([FD + 1, H], m_dt)
        nc.sync.dma_start(gw_sb[:], gw)
        w1bw_sb = cpool.tile([FD + 1, H], m_dt)
        nc.sync.dma_start(w1bw_sb[:], w1bw)
        domE_sb = cpool.tile([FD + 1, spc], m_dt)
        nc.sync.dma_start(domE_sb[:], domE)
        brt_sb = cpool.tile([128, windows * t_tiles], dt.bfloat16)
        nc.sync.dma_start(brt_sb[:], brt)
        out_sb = cpool.tile([1, spc], dt.float32)

        # ---- PE warm-up: ~4.5us of dummy matmuls while DMA prefills, so HAM
        # flips to K=8/8 before the real stream starts.
        warm_t = pm.tile([128, H], dt.float32, tag="pmlp")
        for i in range(44):
            nc.tensor.matmul(
                warm_t[:, 0:128], iotr_sb[:], iotr_sb[:],
                start=(i == 0), stop=(i == 43))

        gstate = {}

        def emit_gamma(wg, span):
            """gammaT for windows [wg, wg+GRP): [128 h, HC, span] bf16."""
            gam = spool.tile([128, HC, NG], m_dt, tag="gam")
            dom_s = domE_sb[:, wg * SEG_W: wg * SEG_W + span]
            for hc in range(HC):
                pgb = pm.tile([128, H], dt.float32, tag="pmlp")
                nc.tensor.matmul(
                    pgb[:, 0:span],
                    gw_sb[:, hc * 128:(hc + 1) * 128], dom_s,
                    start=True, stop=True)
                nc.scalar.copy(gam[:, hc, 0:span], pgb[:, 0:span])
            gstate["gam"] = gam
            gstate["gmodT"] = spool.tile(
                [128, HC, NG], m_dt, tag="gmodT", name="gmodT")

        def emit_mlp(wg, span):
            """MLP for windows [wg, wg+GRP) from gstate['gmodT']."""
            gmodT = gstate["gmodT"]
            dom_s = domE_sb[:, wg * SEG_W: wg * SEG_W + span]
            # layer 1 (+ folded beta/beta_b via K=17 matmul)
            h1 = spool.tile([128, HC, NG], m_dt, tag="h1")
            for jc in range(JC):
                ph1 = pm.tile([128, NG], dt.float32, tag="pmlp")
                nc.tensor.matmul(
                    ph1[:, 0:span],
                    w1bw_sb[:, jc * 128:(jc + 1) * 128], dom_s,
                    start=True, stop=False, skip_group_check=True)
                for hc in range(HC):
                    nc.tensor.matmul(
                        ph1[:, 0:span],
                        w1_sb[:, hc, jc * 128:(jc + 1) * 128],
                        gmodT[:, hc, 0:span],
                        start=False, stop=(hc == HC - 1), skip_group_check=True)
                if SILU_COMPOSE:
                    z1 = spool.tile([128, NG], dt.float32, tag="z1")
                    nc.scalar.activation(
                        z1[:, 0:span], ph1[:, 0:span],
                        mybir.ActivationFunctionType.Identity,
                        bias=b1_sb[:, jc:jc + 1])
                    nc.scalar.activation(
                        h1[:, jc, 0:span], z1[:, 0:span],
                        mybir.ActivationFunctionType.Sigmoid)
                    nc.vector.tensor_mul(
                        h1[:, jc, 0:span], h1[:, jc, 0:span], z1[:, 0:span])
                else:
                    nc.scalar.activation(
                        h1[:, jc, 0:span], ph1[:, 0:span],
                        mybir.ActivationFunctionType.Silu,
                        bias=b1_sb[:, jc:jc + 1])
            # layer 2
            h2 = spool.tile([128, KC, NG], m_dt, tag="h2")
            for kc in range(KC):
                ph2 = pm.tile([128, NG], dt.float32, tag="pmlp")
                for hc in range(HC):
                    nc.tensor.matmul(
                        ph2[:, 0:span],
                        w2_sb[:, hc, kc * 128:(kc + 1) * 128],
                        h1[:, hc, 0:span],
                        start=(hc == 0), stop=(hc == HC - 1))
                if SILU_COMPOSE:
                    z2 = spool.tile([128, NG], dt.float32, tag="z2")
                    nc.scalar.activation(
                        z2[:, 0:span], ph2[:, 0:span],
                        mybir.ActivationFunctionType.Identity,
                        bias=b2_sb[:, kc:kc + 1])
                    nc.scalar.activation(
                        h2[:, kc, 0:span], z2[:, 0:span],
                        mybir.ActivationFunctionType.Sigmoid)
                    nc.vector.tensor_mul(
                        h2[:, kc, 0:span], h2[:, kc, 0:span], z2[:, 0:span])
                else:
                    nc.scalar.activation(
                        h2[:, kc, 0:span], ph2[:, 0:span],
                        mybir.ActivationFunctionType.Silu,
                        bias=b2_sb[:, kc:kc + 1])
            # output head
            po = pm.tile([1, NG], dt.float32, tag="pmlp")
            for kc in range(KC):
                nc.tensor.matmul(
                    po[:, 0:span], w3_sb[:, kc:kc + 1], h2[:, kc, 0:span],
                    start=(kc == 0), stop=(kc == KC - 1))
            nc.scalar.activation(
                out_sb[0:1, wg * SEG_W: wg * SEG_W + span], po[:, 0:span],
                mybir.ActivationFunctionType.Identity,
                bias=b3_sb[0:1, 0:1])

        for w in range(windows):
            if w % GRP == 0:
                emit_gamma(w, min(NG, spc - w * SEG_W))

            # --- segment-sum for this window: accumulate [128 seg, H] ---
            pg_t = pg.tile([128, H], dt.float32)
            off = 0
            for blk in blocks:
                x_sb = xpool.tile([128, XT, H], x_dt)
                nc.sync.dma_start(
                    x_sb[:, 0:blk, :],
                    xp[w][:, off * H:(off + blk) * H]
                    .rearrange("p (c h) -> p c h", c=blk))
                # batched one-hot for the whole block: [128, blk, 128] fp8
                oh = ohpool.tile([128, XT, 128], x_dt)
                iotr_v = iotr_sb[:].rearrange("p (o s) -> p o s", o=1)
                brt_v = brt_sb[:, w * t_tiles + off: w * t_tiles + off + blk]
                brt_v = brt_v.rearrange("p (c o) -> p c o", o=1)
                in0, in1 = bass.broadcast_tensor_aps(iotr_v, brt_v)
                nc.vector.tensor_tensor(oh[:, 0:blk, :], in0, in1, is_eq)
                for gpair in range(blk // 2):
                    ti = off + 2 * gpair
                    nc.tensor.matmul(
                        pg_t[:],
                        oh[:, 2 * gpair:2 * gpair + 2, :],
                        x_sb[:, 2 * gpair:2 * gpair + 2, :],
                        start=(ti == 0), stop=(ti == t_tiles - 2),
                        perf_mode=DR)
                off += blk

            # --- evict g, transpose, FiLM multiply into group gmodT ---
            g_sb = spool.tile([128, H], dt.float32, tag="g")
            nc.scalar.copy(g_sb[:], pg_t[:])
            pt_t = pt.tile([128, H], dt.float32)
            for hc in range(HC):
                nc.tensor.transpose(
                    pt_t[:, hc * 128:(hc + 1) * 128],
                    g_sb[:, hc * 128:(hc + 1) * 128],
                    iden_sb[:])
            wi = w % GRP
            gam = gstate["gam"]
            gmodT = gstate["gmodT"]
            pt_v = pt_t[:].rearrange("p (c s) -> p c s", c=HC)
            gm_v = gmodT[:].rearrange("p c (g s) -> p c g s", g=GRP)
            ga_v = gam[:].rearrange("p c (g s) -> p c g s", g=GRP)
            nc.vector.tensor_mul(
                gm_v[:, :, wi, :], pt_v, ga_v[:, :, wi, :])

            if w % GRP == GRP - 1 or w == windows - 1:
                wg = (w // GRP) * GRP
                emit_mlp(wg, min(NG, spc - wg * SEG_W))

        nc.sync.dma_start(out, out_sb[:])

    nc.compile()
    return nc


def _sigma_delta_fp8(x: np.ndarray, batch: np.ndarray, n_segs: int) -> np.ndarray:
    """fp8-e4m3 quantization of x with per-(segment, h) error feedback.

    Within each segment the quantization errors telescope, so segment sums
    of the returned array match the fp32 sums to ~1 quantum.
    """
    starts = np.searchsorted(batch, np.arange(n_segs + 1))
    lens = np.diff(starts)
    L = int(lens.max())
    xq = np.empty(x.shape, dtype=F8)
    order = np.argsort(-lens, kind="stable")  # longest first: shrinking actives
    sorted_lens = lens[order]
    sorted_starts = starts[order]
    carry = np.zeros((n_segs, x.shape[1]), np.float32)
    for k in range(L):
        n_act = int(np.searchsorted(-sorted_lens, -k, side="left"))
        if n_act == 0:
            break
        rows = sorted_starts[:n_act] + k
        v = x[rows] + carry[:n_act]
        q = v.astype(F8)
        carry[:n_act] = v - q.astype(np.float32)
        xq[rows] = q
    return xq


def _balance_windows(batch: np.ndarray, n_segs: int, n_windows: int):
    """Greedy LPT assignment of segments to windows (128 segments each),
    equalizing node counts.  Returns (win_of_seg, slot_of_seg, t_tiles)."""
    counts = np.bincount(batch, minlength=n_segs)
    order = np.argsort(-counts, kind="stable")
    loads = np.zeros(n_windows, np.int64)
    nseg = np.zeros(n_windows, np.int64)
    win = np.empty(n_segs, np.int64)
    INF = 1 << 40
    for s in order:
        eligible = np.where(nseg < SEG_W, loads, INF)
        w = int(np.argmin(eligible))
        win[s] = w
        loads[w] += counts[s]
        nseg[w] += 1
    assert (nseg == SEG_W).all()
    # slot of each segment within its window (stable by segment id)
    o = np.argsort(win, kind="stable")
    slot = np.empty(n_segs, np.int64)
    slot[o] = np.arange(n_segs) - np.repeat(
        np.arange(n_windows) * SEG_W, SEG_W)
    t_tiles = max(2, 2 * int(-(-loads.max() // 256)))
    return win, slot, t_tiles


def prepare_core_inputs(
    x, batch, domain_emb, gamma_w, gamma_b, beta_w, beta_b,
    w1, b1, w2, b2, w3, b3,
    spc: int, n_cores: int, plan=None,
):
    """Slice/pad/pack the full inputs into one in_map per core.

    Returns (in_maps, seg_pos) where seg_pos[seg] is the segment's position
    in the permuted, concatenated output."""
    n_segs = spc * n_cores
    windows = spc // SEG_W
    n_win_tot = windows * n_cores

    batch = np.ascontiguousarray(np.asarray(batch).astype(np.int64))
    x = np.asarray(x, dtype=np.float32)

    if plan is None:
        plan = _balance_windows(batch, n_segs, n_win_tot)
    win, slot, t_tiles = plan
    npw = SEG_W * t_tiles

    w1_f = np.asarray(w1, np.float32)
    bw_ext = np.concatenate([np.asarray(beta_w, np.float32).T,
                             np.asarray(beta_b, np.float32)[None]], axis=0)  # [17, H]
    w1bw = bw_ext @ w1_f.T                                                   # [17, H]

    shared = {
        "gw": np.ascontiguousarray(_f32_to_bf16(
            np.concatenate([np.asarray(gamma_w, np.float32).T,
                            np.asarray(gamma_b, np.float32)[None]], axis=0))),
        "w1bw": np.ascontiguousarray(_f32_to_bf16(w1bw)),
        "w1t": np.ascontiguousarray(_f32_to_bf16(w1_f.T)),
        "w2t": np.ascontiguousarray(_f32_to_bf16(np.asarray(w2, np.float32).T)),
        "w3c": np.ascontiguousarray(
            _f32_to_bf16(np.asarray(w3, np.float32).reshape(H2 // 128, 128).T)),
        "b1c": np.ascontiguousarray(np.asarray(b1, np.float32).reshape(H // 128, 128).T),
        "b2c": np.ascontiguousarray(np.asarray(b2, np.float32).reshape(H2 // 128, 128).T),
        "b3c": np.asarray(b3, np.float32).reshape(1, 1),
        "iden": np.eye(128, dtype=np.float32),
        "iotr": np.tile(np.arange(128, dtype=np.float32), (128, 1)).astype(BF16),
    }

    xq_u8 = _sigma_delta_fp8(x, batch, n_segs).view(np.uint8)

    # permuted node order: grouped by window (stable, so per-segment runs stay
    # contiguous), with per-node window/slot ids
    node_win = win[batch]
    node_slot = slot[batch].astype(np.float32)
    order = np.argsort(node_win, kind="stable")
    wstarts = np.searchsorted(node_win[order], np.arange(n_win_tot + 1))

    # segment position in the permuted output
    seg_pos = win * SEG_W + slot

    dom = np.asarray(domain_emb, np.float32)
    dom_ext = np.concatenate([dom.T, np.ones((1, n_segs), np.float32)], axis=0)
    domP = np.empty((FD + 1, n_segs), np.float32)
    domP[:, seg_pos] = dom_ext

    in_maps = []
    for core in range(n_cores):
        xp_c = np.zeros((windows, npw, H), dtype=np.uint8)
        brt_c = np.full((windows, npw), -1024.0, dtype=np.float32)
        for wl in range(windows):
            wg = core * windows + wl
            ns = order[wstarts[wg]:wstarts[wg + 1]]
            cnt = len(ns)
            if cnt > npw:
                raise ValueError(f"window overflow: {cnt} > {npw}")
            if cnt == 0:
                continue
            xp_c[wl, :cnt] = xq_u8[ns]
            brt_c[wl, :cnt] = node_slot[ns]
        # [windows, npw, H] -> [windows, 128, t_tiles*H]: node c*128+p at
        # partition p, free slot (c, h)
        xp_c = np.ascontiguousarray(
            xp_c.reshape(windows, t_tiles, 128, H)
            .transpose(0, 2, 1, 3)
            .reshape(windows, 128, t_tiles * H)).view(F8)
        # [windows, npw] -> [128, windows*t_tiles]: brt[p, w*t_tiles+ti]
        brt_c = np.ascontiguousarray(
            brt_c.reshape(windows, t_tiles, 128).transpose(2, 0, 1)
            .reshape(128, windows * t_tiles).astype(BF16))
        domE_c = np.ascontiguousarray(
            _f32_to_bf16(domP[:, core * spc:(core + 1) * spc]))
        in_maps.append({"xp": xp_c, "brt": brt_c, "domE": domE_c, **shared})
    return in_maps, seg_pos, t_tiles


_PROGRAM_CACHE: dict = {}

# Set by test harnesses: request an NTFF trace and stash the raw results.
TRACE = False
LAST_RESULT = None


def kernel(**inputs) -> np.ndarray:
    x = np.asarray(inputs["x"], dtype=np.float32)
    batch = np.ascontiguousarray(np.asarray(inputs["batch"]).astype(np.int64))
    assert x.shape == (N_NODES, H), x.shape

    spc = B_SEGS // N_CORES

    in_maps, seg_pos, t_tiles = prepare_core_inputs(
        x, batch,
        inputs["domain_emb"], inputs["gamma_w"], inputs["gamma_b"],
        inputs["beta_w"], inputs["beta_b"],
        inputs["w1"], inputs["b1"], inputs["w2"], inputs["b2"],
        inputs["w3"], inputs["b3"],
        spc, N_CORES,
    )

    key = (spc, t_tiles, N_CORES)
    if key not in _PROGRAM_CACHE:
        _PROGRAM_CACHE[key] = build_program(spc, t_tiles, N_CORES)
    nc = _PROGRAM_CACHE[key]

    res = bass_utils.run_bass_kernel_spmd(
        nc, in_maps, core_ids=list(range(N_CORES)), trace=TRACE)
    global LAST_RESULT
    LAST_RESULT = res
    out_perm = np.concatenate(
        [res.results[c]["out"].reshape(-1) for c in range(N_CORES)])
    return np.ascontiguousarray(out_perm[seg_pos].astype(np.float32))


# revision 10
# speedup vs baseline: 2.3850x; 1.0008x over previous
"""Trainium2 Bass kernel for nn_GemNetOutput (segment_reduce + FiLM + MLP head).

Reference computation (all fp32):
    g     = segment_sum(x, batch, num_segments=B)        # [B, H]
    gamma = domain_emb @ gamma_w.T + gamma_b             # [B, H]
    beta  = domain_emb @ beta_w.T  + beta_b              # [B, H]
    g     = gamma * g + beta
    h     = silu(g @ w1.T + b1)                          # [B, H]
    h     = silu(h @ w2.T + b2)                          # [B, H/2]
    out   = (h @ w3.T + b3).squeeze(-1)                  # [B]

Shapes: N=1e6 nodes, B=16384 graphs, H=512, FD=16.  `batch` is SORTED.

Strategy (8 NeuronCores, no collectives needed):
  - The 16384 segments are BIN-PACKED into 128 windows of exactly 128
    segments each, equalizing the node count per window (greedy LPT on the
    host).  Core c owns windows [c*16, (c+1)*16).  All cores run one
    identical static program; every window is padded to the same t_tiles
    node tiles (sentinel one-hot ids mask the padding) -- balancing makes
    that padding ~1.5% instead of ~5%.
  - x is converted to fp8-e4m3 on the host with SIGMA-DELTA (error-feedback)
    rounding along each segment: quantization errors telescope within a
    segment, so the device's segment sums match the fp32 sums to ~1 quantum
    instead of sqrt(n) quanta.  Halves HBM traffic vs bf16.
  - x is packed on the host into [window, 128, t_tiles*H] so each DMA block
    is a fully contiguous ~1 MB transfer (8 KB per partition line).
  - segment_sum on the PE with fp8 DoubleRow matmuls: each matmul consumes
    TWO 128-node tiles (contraction 256) against a [128, 2, 128] one-hot.
  - One-hot built on DVE with one batched tensor_tensor per DMA block
    (stride-0 broadcast APs) instead of one tensor_scalar per tile.
  - beta (incl. beta_b) is folded into the MLP-1 accumulation on the host:
    ph1 += (W1 @ beta_w_ext.T) @ dom_ext, one small K=17 matmul per j-chunk.
  - FiLM multiply + MLP run per GROUP of 4 windows in transposed
    [feature, seg] layout so MLP matmuls have N=512 moving operands.
"""

import sys
from contextlib import ExitStack

for _p in ("/opt/trn_rl_repo", "/opt/pypackages"):
    if _p not in sys.path:
        sys.path.append(_p)

import ml_dtypes
import numpy as np

import concourse.bass as bass
import concourse.tile as tile
from concourse import bacc, mybir
from concourse import bass_utils

dt = mybir.dt

# Problem constants (hardcoded per the contract).
N_NODES = 1_000_000
B_SEGS = 16_384
H = 512
H2 = 256
FD = 16
N_CORES = 8
SEG_W = 128          # segments per window (PSUM partition dim)
GRP = 4              # windows per MLP group (moving N = GRP*SEG_W = 512)
XT = 16              # max node subtiles (of 128 rows) per x DMA block

BF16 = ml_dtypes.bfloat16
F8 = ml_dtypes.float8_e4m3fn

# CoreSim has no Silu LUT; compose silu = z * sigmoid(z) when True (sim tests).
SILU_COMPOSE = False


def _f32_to_bf16(a: np.ndarray) -> np.ndarray:
    return np.ascontiguousarray(a, dtype=np.float32).astype(BF16)


def _blocks_of(t_tiles: int) -> tuple:
    """Split t_tiles into even-sized DMA blocks of at most XT subtiles."""
    blocks = [XT] * (t_tiles // XT)
    rem = t_tiles % XT
    if rem:
        blocks.append(rem)
    assert all(b % 2 == 0 for b in blocks)
    return tuple(blocks)


def build_program(spc: int, t_tiles: int, n_cores: int):
    """Build the per-core Bass/Tile program.

    spc: segments per core (multiple of 128)
    t_tiles: node tiles (of 128) per 128-segment window; even
    """
    assert t_tiles % 2 == 0
    windows = spc // SEG_W
    blocks = _blocks_of(t_tiles)
    x_dt = dt.float8e4
    m_dt = dt.bfloat16             # MLP matmul dtype

    nc = bacc.Bacc(
        "TRN2",
        target_bir_lowering=False,
        debug=False,
        enable_asserts=False,
        num_devices=n_cores,
    )

    xp = nc.dram_tensor("xp", [windows, 128, t_tiles * H], x_dt,
                        kind="ExternalInput").ap()
    brt = nc.dram_tensor("brt", [128, windows * t_tiles], dt.bfloat16,
                         kind="ExternalInput").ap()
    domE = nc.dram_tensor("domE", [FD + 1, spc], m_dt, kind="ExternalInput").ap()
    gw = nc.dram_tensor("gw", [FD + 1, H], m_dt, kind="ExternalInput").ap()
    w1bw = nc.dram_tensor("w1bw", [FD + 1, H], m_dt, kind="ExternalInput").ap()
    w1t = nc.dram_tensor("w1t", [H, H], m_dt, kind="ExternalInput").ap()
    w2t = nc.dram_tensor("w2t", [H, H2], m_dt, kind="ExternalInput").ap()
    w3c = nc.dram_tensor("w3c", [128, H2 // 128], m_dt, kind="ExternalInput").ap()
    b1c = nc.dram_tensor("b1c", [128, H // 128], dt.float32, kind="ExternalInput").ap()
    b2c = nc.dram_tensor("b2c", [128, H2 // 128], dt.float32, kind="ExternalInput").ap()
    b3c = nc.dram_tensor("b3c", [1, 1], dt.float32, kind="ExternalInput").ap()
    iden = nc.dram_tensor("iden", [128, 128], dt.float32, kind="ExternalInput").ap()
    iotr = nc.dram_tensor("iotr", [128, 128], dt.bfloat16, kind="ExternalInput").ap()
    out = nc.dram_tensor("out", [1, spc], dt.float32, kind="ExternalOutput").ap()

    HC = H // 128       # 4 h-chunks
    JC = H // 128       # 4 layer-1 output chunks
    KC = H2 // 128      # 2 layer-2 output chunks
    NG = GRP * SEG_W    # moving width of group-level MLP matmuls

    is_eq = mybir.AluOpType.is_equal
    DR = mybir.MatmulPerfMode.DoubleRow

    with tile.TileContext(nc) as tc, ExitStack() as ctx:
        cpool = ctx.enter_context(tc.tile_pool(name="consts", bufs=1))
        xpool = ctx.enter_context(tc.tile_pool(name="x", bufs=8))
        ohpool = ctx.enter_context(tc.tile_pool(name="oh", bufs=6))
        spool = ctx.enter_context(tc.tile_pool(name="work", bufs=2))
        pg = ctx.enter_context(tc.tile_pool(name="pg", bufs=2, space=bass.MemorySpace.PSUM))
        pt = ctx.enter_context(tc.tile_pool(name="pt", bufs=2, space=bass.MemorySpace.PSUM))
        pm = ctx.enter_context(tc.tile_pool(name="pm", bufs=3, space=bass.MemorySpace.PSUM))

        # ---- constants / weights into SBUF ----
        iden_sb = cpool.tile([128, 128], dt.float32)
        nc.sync.dma_start(iden_sb[:], iden)
        iotr_sb = cpool.tile([128, 128], dt.bfloat16)
        nc.sync.dma_start(iotr_sb[:], iotr)
        w1_sb = cpool.tile([128, HC, H], m_dt)
        w2_sb = cpool.tile([128, HC, H2], m_dt)
        w3_sb = cpool.tile([128, KC], m_dt)
        b1_sb = cpool.tile([128, JC], dt.float32)
        b2_sb = cpool.tile([128, KC], dt.float32)
        b3_sb = cpool.tile([1, 1], dt.float32)

        def emit_weight_dmas():
            nc.sync.dma_start(w1_sb[:], w1t.rearrange("(c p) j -> p c j", p=128))
            nc.sync.dma_start(w2_sb[:], w2t.rearrange("(c p) j -> p c j", p=128))
            nc.sync.dma_start(w3_sb[:], w3c)
            nc.sync.dma_start(b1_sb[:], b1c)
            nc.sync.dma_start(b2_sb[:], b2c)
            nc.sync.dma_start(b3_sb[:], b3c)
        gw_sb = cpool.tile([FD + 1, H], m_dt)
        nc.sync.dma_start(gw_sb[:], gw)
        w1bw_sb = cpool.tile([FD + 1, H], m_dt)
        nc.sync.dma_start(w1bw_sb[:], w1bw)
        domE_sb = cpool.tile([FD + 1, spc], m_dt)
        nc.sync.dma_start(domE_sb[:], domE)
        brt_sb = cpool.tile([128, windows * t_tiles], dt.bfloat16)
        nc.sync.dma_start(brt_sb[:], brt)
        out_sb = cpool.tile([1, spc], dt.float32)

        # ---- PE warm-up: ~4.5us of dummy matmuls while DMA prefills, so HAM
        # flips to K=8/8 before the real stream starts.
        warm_t = pm.tile([128, H], dt.float32, tag="pmlp")
        for i in range(44):
            nc.tensor.matmul(
                warm_t[:, 0:128], iotr_sb[:], iotr_sb[:],
                start=(i == 0), stop=(i == 43))

        gstate = {}

        def emit_gamma(wg, span):
            """gammaT for windows [wg, wg+GRP): [128 h, HC, span] bf16."""
            gam = spool.tile([128, HC, NG], m_dt, tag="gam")
            dom_s = domE_sb[:, wg * SEG_W: wg * SEG_W + span]
            for hc in range(HC):
                pgb = pm.tile([128, H], dt.float32, tag="pmlp")
                nc.tensor.matmul(
                    pgb[:, 0:span],
                    gw_sb[:, hc * 128:(hc + 1) * 128], dom_s,
                    start=True, stop=True)
                nc.scalar.copy(gam[:, hc, 0:span], pgb[:, 0:span])
            gstate[wg // GRP] = (gam, spool.tile(
                [128, HC, NG], m_dt, tag="gmodT", name="gmodT"))

        def emit_mlp(wg, span):
            """MLP for windows [wg, wg+GRP)."""
            gmodT = gstate[wg // GRP][1]
            dom_s = domE_sb[:, wg * SEG_W: wg * SEG_W + span]
            # layer 1 (+ folded beta/beta_b via K=17 matmul)
            h1 = spool.tile([128, HC, NG], m_dt, tag="h1")
            for jc in range(JC):
                ph1 = pm.tile([128, NG], dt.float32, tag="pmlp")
                nc.tensor.matmul(
                    ph1[:, 0:span],
                    w1bw_sb[:, jc * 128:(jc + 1) * 128], dom_s,
                    start=True, stop=False, skip_group_check=True)
                for hc in range(HC):
                    nc.tensor.matmul(
                        ph1[:, 0:span],
                        w1_sb[:, hc, jc * 128:(jc + 1) * 128],
                        gmodT[:, hc, 0:span],
                        start=False, stop=(hc == HC - 1), skip_group_check=True)
                if SILU_COMPOSE:
                    z1 = spool.tile([128, NG], dt.float32, tag="z1")
                    nc.scalar.activation(
                        z1[:, 0:span], ph1[:, 0:span],
                        mybir.ActivationFunctionType.Identity,
                        bias=b1_sb[:, jc:jc + 1])
                    nc.scalar.activation(
                        h1[:, jc, 0:span], z1[:, 0:span],
                        mybir.ActivationFunctionType.Sigmoid)
                    nc.vector.tensor_mul(
                        h1[:, jc, 0:span], h1[:, jc, 0:span], z1[:, 0:span])
                else:
                    nc.scalar.activation(
                        h1[:, jc, 0:span], ph1[:, 0:span],
                        mybir.ActivationFunctionType.Silu,
                        bias=b1_sb[:, jc:jc + 1])
            # layer 2
            h2 = spool.tile([128, KC, NG], m_dt, tag="h2")
            for kc in range(KC):
                ph2 = pm.tile([128, NG], dt.float32, tag="pmlp")
                for hc in range(HC):
                    nc.tensor.matmul(
                        ph2[:, 0:span],
                        w2_sb[:, hc, kc * 128:(kc + 1) * 128],
                        h1[:, hc, 0:span],
                        start=(hc == 0), stop=(hc == HC - 1))
                if SILU_COMPOSE:
                    z2 = spool.tile([128, NG], dt.float32, tag="z2")
                    nc.scalar.activation(
                        z2[:, 0:span], ph2[:, 0:span],
                        mybir.ActivationFunctionType.Identity,
                        bias=b2_sb[:, kc:kc + 1])
                    nc.scalar.activation(
                        h2[:, kc, 0:span], z2[:, 0:span],
                        mybir.ActivationFunctionType.Sigmoid)
                    nc.vector.tensor_mul(
                        h2[:, kc, 0:span], h2[:, kc, 0:span], z2[:, 0:span])
                else:
                    nc.scalar.activation(
                        h2[:, kc, 0:span], ph2[:, 0:span],
                        mybir.ActivationFunctionType.Silu,
                        bias=b2_sb[:, kc:kc + 1])
            # output head
            po = pm.tile([1, NG], dt.float32, tag="pmlp")
            for kc in range(KC):
                nc.tensor.matmul(
                    po[:, 0:span], w3_sb[:, kc:kc + 1], h2[:, kc, 0:span],
                    start=(kc == 0), stop=(kc == KC - 1))
            nc.scalar.activation(
                out_sb[0:1, wg * SEG_W: wg * SEG_W + span], po[:, 0:span],
                mybir.ActivationFunctionType.Identity,
                bias=b3_sb[0:1, 0:1])

        def emit_epilogue(w, pg_t):
            """Evict g(w), transpose, FiLM multiply into its group's gmodT."""
            g_sb = spool.tile([128, H], dt.float32, tag="g", name="g_sb")
            nc.scalar.copy(g_sb[:], pg_t[:])
            pt_t = pt.tile([128, H], dt.float32, name="pt_t")
            for hc in range(HC):
                nc.tensor.transpose(
                    pt_t[:, hc * 128:(hc + 1) * 128],
                    g_sb[:, hc * 128:(hc + 1) * 128],
                    iden_sb[:])
            wi = w % GRP
            gam, gmodT = gstate[w // GRP]
            pt_v = pt_t[:].rearrange("p (c s) -> p c s", c=HC)
            gm_v = gmodT[:].rearrange("p c (g s) -> p c g s", g=GRP)
            ga_v = gam[:].rearrange("p c (g s) -> p c g s", g=GRP)
            nc.vector.tensor_mul(
                gm_v[:, :, wi, :], pt_v, ga_v[:, :, wi, :])
            if w % GRP == GRP - 1 or w == windows - 1:
                wg = (w // GRP) * GRP
                emit_mlp(wg, min(NG, spc - wg * SEG_W))

        pend = None   # (window, pg tile) awaiting its epilogue
        for w in range(windows):
            if w % GRP == 0:
                emit_gamma(w, min(NG, spc - w * SEG_W))

            # --- segment-sum for this window: accumulate [128 seg, H] ---
            pg_t = pg.tile([128, H], dt.float32)
            off = 0
            for blk in blocks:
                x_sb = xpool.tile([128, XT, H], x_dt)
                nc.sync.dma_start(
                    x_sb[:, 0:blk, :],
                    xp[w][:, off * H:(off + blk) * H]
                    .rearrange("p (c h) -> p c h", c=blk))
                # batched one-hot for the whole block: [128, blk, 128] fp8
                oh = ohpool.tile([128, XT, 128], x_dt)
                iotr_v = iotr_sb[:].rearrange("p (o s) -> p o s", o=1)
                brt_v = brt_sb[:, w * t_tiles + off: w * t_tiles + off + blk]
                brt_v = brt_v.rearrange("p (c o) -> p c o", o=1)
                in0, in1 = bass.broadcast_tensor_aps(iotr_v, brt_v)
                nc.vector.tensor_tensor(oh[:, 0:blk, :], in0, in1, is_eq)
                for gpair in range(blk // 2):
                    ti = off + 2 * gpair
                    nc.tensor.matmul(
                        pg_t[:],
                        oh[:, 2 * gpair:2 * gpair + 2, :],
                        x_sb[:, 2 * gpair:2 * gpair + 2, :],
                        start=(ti == 0), stop=(ti == t_tiles - 2),
                        perf_mode=DR)
                off += blk

            if w == 0:
                emit_weight_dmas()
            if pend is not None:
                emit_epilogue(*pend)
            pend = (w, pg_t)

        emit_epilogue(*pend)
        nc.sync.dma_start(out, out_sb[:])

    nc.compile()
    return nc


def _sigma_delta_fp8(x: np.ndarray, batch: np.ndarray, n_segs: int) -> np.ndarray:
    """fp8-e4m3 quantization of x with per-(segment, h) error feedback.

    Within each segment the quantization errors telescope, so segment sums
    of the returned array match the fp32 sums to ~1 quantum.
    """
    starts = np.searchsorted(batch, np.arange(n_segs + 1))
    lens = np.diff(starts)
    L = int(lens.max())
    xq = np.empty(x.shape, dtype=F8)
    order = np.argsort(-lens, kind="stable")  # longest first: shrinking actives
    sorted_lens = lens[order]
    sorted_starts = starts[order]
    carry = np.zeros((n_segs, x.shape[1]), np.float32)
    for k in range(L):
        n_act = int(np.searchsorted(-sorted_lens, -k, side="left"))
        if n_act == 0:
            break
        rows = sorted_starts[:n_act] + k
        v = x[rows] + carry[:n_act]
        q = v.astype(F8)
        carry[:n_act] = v - q.astype(np.float32)
        xq[rows] = q
    return xq


def _balance_windows(batch: np.ndarray, n_segs: int, n_windows: int):
    """Greedy LPT assignment of segments to windows (128 segments each),
    equalizing node counts.  Returns (win_of_seg, slot_of_seg, t_tiles)."""
    counts = np.bincount(batch, minlength=n_segs)
    order = np.argsort(-counts, kind="stable")
    loads = np.zeros(n_windows, np.int64)
    nseg = np.zeros(n_windows, np.int64)
    win = np.empty(n_segs, np.int64)
    INF = 1 << 40
    for s in order:
        eligible = np.where(nseg < SEG_W, loads, INF)
        w = int(np.argmin(eligible))
        win[s] = w
        loads[w] += counts[s]
        nseg[w] += 1
    assert (nseg == SEG_W).all()
    # slot of each segment within its window (stable by segment id)
    o = np.argsort(win, kind="stable")
    slot = np.empty(n_segs, np.int64)
    slot[o] = np.arange(n_segs) - np.repeat(
        np.arange(n_windows) * SEG_W, SEG_W)
    t_tiles = max(2, 2 * int(-(-loads.max() // 256)))
    return win, slot, t_tiles


def prepare_core_inputs(
    x, batch, domain_emb, gamma_w, gamma_b, beta_w, beta_b,
    w1, b1, w2, b2, w3, b3,
    spc: int, n_cores: int, plan=None,
):
    """Slice/pad/pack the full inputs into one in_map per core.

    Returns (in_maps, seg_pos) where seg_pos[seg] is the segment's position
    in the permuted, concatenated output."""
    n_segs = spc * n_cores
    windows = spc // SEG_W
    n_win_tot = windows * n_cores

    batch = np.ascontiguousarray(np.asarray(batch).astype(np.int64))
    x = np.asarray(x, dtype=np.float32)

    if plan is None:
        plan = _balance_windows(batch, n_segs, n_win_tot)
    win, slot, t_tiles = plan
    npw = SEG_W * t_tiles

    w1_f = np.asarray(w1, np.float32)
    bw_ext = np.concatenate([np.asarray(beta_w, np.float32).T,
                             np.asarray(beta_b, np.float32)[None]], axis=0)  # [17, H]
    w1bw = bw_ext @ w1_f.T                                                   # [17, H]

    shared = {
        "gw": np.ascontiguousarray(_f32_to_bf16(
            np.concatenate([np.asarray(gamma_w, np.float32).T,
                            np.asarray(gamma_b, np.float32)[None]], axis=0))),
        "w1bw": np.ascontiguousarray(_f32_to_bf16(w1bw)),
        "w1t": np.ascontiguousarray(_f32_to_bf16(w1_f.T)),
        "w2t": np.ascontiguousarray(_f32_to_bf16(np.asarray(w2, np.float32).T)),
        "w3c": np.ascontiguousarray(
            _f32_to_bf16(np.asarray(w3, np.float32).reshape(H2 // 128, 128).T)),
        "b1c": np.ascontiguousarray(np.asarray(b1, np.float32).reshape(H // 128, 128).T),
        "b2c": np.ascontiguousarray(np.asarray(b2, np.float32).reshape(H2 // 128, 128).T),
        "b3c": np.asarray(b3, np.float32).reshape(1, 1),
        "iden": np.eye(128, dtype=np.float32),
        "iotr": np.tile(np.arange(128, dtype=np.float32), (128, 1)).astype(BF16),
    }

    xq_u8 = _sigma_delta_fp8(x, batch, n_segs).view(np.uint8)

    # permuted node order: grouped by window (stable, so per-segment runs stay
    # contiguous), with per-node window/slot ids
    node_win = win[batch]
    node_slot = slot[batch].astype(np.float32)
    order = np.argsort(node_win, kind="stable")
    wstarts = np.searchsorted(node_win[order], np.arange(n_win_tot + 1))

    # segment position in the permuted output
    seg_pos = win * SEG_W + slot

    dom = np.asarray(domain_emb, np.float32)
    dom_ext = np.concatenate([dom.T, np.ones((1, n_segs), np.float32)], axis=0)
    domP = np.empty((FD + 1, n_segs), np.float32)
    domP[:, seg_pos] = dom_ext

    in_maps = []
    for core in range(n_cores):
        xp_c = np.zeros((windows, npw, H), dtype=np.uint8)
        brt_c = np.full((windows, npw), -1024.0, dtype=np.float32)
        for wl in range(windows):
            wg = core * windows + wl
            ns = order[wstarts[wg]:wstarts[wg + 1]]
            cnt = len(ns)
            if cnt > npw:
                raise ValueError(f"window overflow: {cnt} > {npw}")
            if cnt == 0:
                continue
            xp_c[wl, :cnt] = xq_u8[ns]
            brt_c[wl, :cnt] = node_slot[ns]
        # [windows, npw, H] -> [windows, 128, t_tiles*H]: node c*128+p at
        # partition p, free slot (c, h)
        xp_c = np.ascontiguousarray(
            xp_c.reshape(windows, t_tiles, 128, H)
            .transpose(0, 2, 1, 3)
            .reshape(windows, 128, t_tiles * H)).view(F8)
        # [windows, npw] -> [128, windows*t_tiles]: brt[p, w*t_tiles+ti]
        brt_c = np.ascontiguousarray(
            brt_c.reshape(windows, t_tiles, 128).transpose(2, 0, 1)
            .reshape(128, windows * t_tiles).astype(BF16))
        domE_c = np.ascontiguousarray(
            _f32_to_bf16(domP[:, core * spc:(core + 1) * spc]))
        in_maps.append({"xp": xp_c, "brt": brt_c, "domE": domE_c, **shared})
    return in_maps, seg_pos, t_tiles


_PROGRAM_CACHE: dict = {}

# Set by test harnesses: request an NTFF trace and stash the raw results.
TRACE = False
LAST_RESULT = None


def kernel(**inputs) -> np.ndarray:
    x = np.asarray(inputs["x"], dtype=np.float32)
    batch = np.ascontiguousarray(np.asarray(inputs["batch"]).astype(np.int64))
    assert x.shape == (N_NODES, H), x.shape

    spc = B_SEGS // N_CORES

    in_maps, seg_pos, t_tiles = prepare_core_inputs(
        x, batch,
        inputs["domain_emb"], inputs["gamma_w"], inputs["gamma_b"],
        inputs["beta_w"], inputs["beta_b"],
        inputs["w1"], inputs["b1"], inputs["w2"], inputs["b2"],
        inputs["w3"], inputs["b3"],
        spc, N_CORES,
    )

    key = (spc, t_tiles, N_CORES)
    if key not in _PROGRAM_CACHE:
        _PROGRAM_CACHE[key] = build_program(spc, t_tiles, N_CORES)
    nc = _PROGRAM_CACHE[key]

    res = bass_utils.run_bass_kernel_spmd(
        nc, in_maps, core_ids=list(range(N_CORES)), trace=TRACE)
    global LAST_RESULT
    LAST_RESULT = res
    out_perm = np.concatenate(
        [res.results[c]["out"].reshape(-1) for c in range(N_CORES)])
    return np.ascontiguousarray(out_perm[seg_pos].astype(np.float32))


# revision 11
# speedup vs baseline: 2.3905x; 1.0023x over previous
"""Trainium2 Bass kernel for nn_GemNetOutput (segment_reduce + FiLM + MLP head).

Reference computation (all fp32):
    g     = segment_sum(x, batch, num_segments=B)        # [B, H]
    gamma = domain_emb @ gamma_w.T + gamma_b             # [B, H]
    beta  = domain_emb @ beta_w.T  + beta_b              # [B, H]
    g     = gamma * g + beta
    h     = silu(g @ w1.T + b1)                          # [B, H]
    h     = silu(h @ w2.T + b2)                          # [B, H/2]
    out   = (h @ w3.T + b3).squeeze(-1)                  # [B]

Shapes: N=1e6 nodes, B=16384 graphs, H=512, FD=16.  `batch` is SORTED.

Strategy (8 NeuronCores, no collectives needed):
  - The 16384 segments are BIN-PACKED into 128 windows of exactly 128
    segments each, equalizing the node count per window (greedy LPT on the
    host).  Core c owns windows [c*16, (c+1)*16).  All cores run one
    identical static program; every window is padded to the same t_tiles
    node tiles (sentinel one-hot ids mask the padding) -- balancing makes
    that padding ~1.5% instead of ~5%.
  - x is converted to fp8-e4m3 on the host with SIGMA-DELTA (error-feedback)
    rounding along each segment: quantization errors telescope within a
    segment, so the device's segment sums match the fp32 sums to ~1 quantum
    instead of sqrt(n) quanta.  Halves HBM traffic vs bf16.
  - x is packed on the host into [window, 128, t_tiles*H] so each DMA block
    is a fully contiguous ~1 MB transfer (8 KB per partition line).
  - segment_sum on the PE with fp8 DoubleRow matmuls: each matmul consumes
    TWO 128-node tiles (contraction 256) against a [128, 2, 128] one-hot.
  - One-hot built on DVE with one batched tensor_tensor per DMA block
    (stride-0 broadcast APs) instead of one tensor_scalar per tile.
  - beta (incl. beta_b) is folded into the MLP-1 accumulation on the host:
    ph1 += (W1 @ beta_w_ext.T) @ dom_ext, one small K=17 matmul per j-chunk.
  - FiLM multiply + MLP run per GROUP of 4 windows in transposed
    [feature, seg] layout so MLP matmuls have N=512 moving operands.
"""

import sys
from contextlib import ExitStack

for _p in ("/opt/trn_rl_repo", "/opt/pypackages"):
    if _p not in sys.path:
        sys.path.append(_p)

import ml_dtypes
import numpy as np

import concourse.bass as bass
import concourse.tile as tile
from concourse import bacc, mybir
from concourse import bass_utils

dt = mybir.dt

# Problem constants (hardcoded per the contract).
N_NODES = 1_000_000
B_SEGS = 16_384
H = 512
H2 = 256
FD = 16
N_CORES = 8
SEG_W = 128          # segments per window (PSUM partition dim)
GRP = 4              # windows per MLP group (moving N = GRP*SEG_W = 512)
XT = 16              # max node subtiles (of 128 rows) per x DMA block

BF16 = ml_dtypes.bfloat16
F8 = ml_dtypes.float8_e4m3fn

# CoreSim has no Silu LUT; compose silu = z * sigmoid(z) when True (sim tests).
SILU_COMPOSE = False


def _f32_to_bf16(a: np.ndarray) -> np.ndarray:
    return np.ascontiguousarray(a, dtype=np.float32).astype(BF16)


def _blocks_of(t_tiles: int) -> tuple:
    """Split t_tiles into even-sized DMA blocks of at most XT subtiles."""
    blocks = [XT] * (t_tiles // XT)
    rem = t_tiles % XT
    if rem:
        blocks.append(rem)
    assert all(b % 2 == 0 for b in blocks)
    return tuple(blocks)


def build_program(spc: int, t_tiles: int, n_cores: int):
    """Build the per-core Bass/Tile program.

    spc: segments per core (multiple of 128)
    t_tiles: node tiles (of 128) per 128-segment window; even
    """
    assert t_tiles % 2 == 0
    windows = spc // SEG_W
    blocks = _blocks_of(t_tiles)
    x_dt = dt.float8e4
    m_dt = dt.bfloat16             # MLP matmul dtype

    nc = bacc.Bacc(
        "TRN2",
        target_bir_lowering=False,
        debug=False,
        enable_asserts=False,
        num_devices=n_cores,
    )

    xp = nc.dram_tensor("xp", [windows, 128, t_tiles * H], x_dt,
                        kind="ExternalInput").ap()
    brt = nc.dram_tensor("brt", [128, windows * t_tiles], dt.bfloat16,
                         kind="ExternalInput").ap()
    domE = nc.dram_tensor("domE", [FD + 1, spc], m_dt, kind="ExternalInput").ap()
    gw = nc.dram_tensor("gw", [FD + 1, H], m_dt, kind="ExternalInput").ap()
    w1bw = nc.dram_tensor("w1bw", [FD + 1, H], m_dt, kind="ExternalInput").ap()
    w1t = nc.dram_tensor("w1t", [H, H], m_dt, kind="ExternalInput").ap()
    w2t = nc.dram_tensor("w2t", [H, H2], m_dt, kind="ExternalInput").ap()
    w3c = nc.dram_tensor("w3c", [128, H2 // 128], m_dt, kind="ExternalInput").ap()
    b1c = nc.dram_tensor("b1c", [128, H // 128], dt.float32, kind="ExternalInput").ap()
    b2c = nc.dram_tensor("b2c", [128, H2 // 128], dt.float32, kind="ExternalInput").ap()
    b3c = nc.dram_tensor("b3c", [1, 1], dt.float32, kind="ExternalInput").ap()
    iden = nc.dram_tensor("iden", [128, 128], dt.float32, kind="ExternalInput").ap()
    iotr = nc.dram_tensor("iotr", [128, 128], dt.bfloat16, kind="ExternalInput").ap()
    out = nc.dram_tensor("out", [1, spc], dt.float32, kind="ExternalOutput").ap()

    HC = H // 128       # 4 h-chunks
    JC = H // 128       # 4 layer-1 output chunks
    KC = H2 // 128      # 2 layer-2 output chunks
    NG = GRP * SEG_W    # moving width of group-level MLP matmuls

    is_eq = mybir.AluOpType.is_equal
    DR = mybir.MatmulPerfMode.DoubleRow

    with tile.TileContext(nc) as tc, ExitStack() as ctx:
        cpool = ctx.enter_context(tc.tile_pool(name="consts", bufs=1))
        xpool = ctx.enter_context(tc.tile_pool(name="x", bufs=8))
        ohpool = ctx.enter_context(tc.tile_pool(name="oh", bufs=6))
        spool = ctx.enter_context(tc.tile_pool(name="work", bufs=2))
        pg = ctx.enter_context(tc.tile_pool(name="pg", bufs=2, space=bass.MemorySpace.PSUM))
        pt = ctx.enter_context(tc.tile_pool(name="pt", bufs=2, space=bass.MemorySpace.PSUM))
        pm = ctx.enter_context(tc.tile_pool(name="pm", bufs=3, space=bass.MemorySpace.PSUM))

        # ---- constants / weights into SBUF ----
        iden_sb = cpool.tile([128, 128], dt.float32)
        nc.sync.dma_start(iden_sb[:], iden)
        iotr_sb = cpool.tile([128, 128], dt.bfloat16)
        nc.sync.dma_start(iotr_sb[:], iotr)
        w1_sb = cpool.tile([128, HC, H], m_dt)
        w2_sb = cpool.tile([128, HC, H2], m_dt)
        w3_sb = cpool.tile([128, KC], m_dt)
        b1_sb = cpool.tile([128, JC], dt.float32)
        b2_sb = cpool.tile([128, KC], dt.float32)
        b3_sb = cpool.tile([1, 1], dt.float32)

        def emit_weight_dmas():
            nc.sync.dma_start(w1_sb[:], w1t.rearrange("(c p) j -> p c j", p=128))
            nc.sync.dma_start(w2_sb[:], w2t.rearrange("(c p) j -> p c j", p=128))
            nc.sync.dma_start(w3_sb[:], w3c)
            nc.sync.dma_start(b1_sb[:], b1c)
            nc.sync.dma_start(b2_sb[:], b2c)
            nc.sync.dma_start(b3_sb[:], b3c)
        gw_sb = cpool.tile([FD + 1, H], m_dt)
        nc.sync.dma_start(gw_sb[:], gw)
        w1bw_sb = cpool.tile([FD + 1, H], m_dt)
        nc.sync.dma_start(w1bw_sb[:], w1bw)
        domE_sb = cpool.tile([FD + 1, spc], m_dt)
        nc.sync.dma_start(domE_sb[:], domE)
        brt_sb = cpool.tile([128, windows * t_tiles], dt.bfloat16)
        nc.sync.dma_start(brt_sb[:], brt)
        out_sb = cpool.tile([1, spc], dt.float32)

        # ---- PE warm-up: ~4.5us of dummy matmuls while DMA prefills, so HAM
        # flips to K=8/8 before the real stream starts.
        warm_t = pm.tile([128, H], dt.float32, tag="pmlp")
        for i in range(44):
            nc.tensor.matmul(
                warm_t[:, 0:128], iotr_sb[:], iotr_sb[:],
                start=(i == 0), stop=(i == 43))

        gstate = {}

        def emit_gamma(wg, span):
            """gammaT for windows [wg, wg+GRP): [128 h, HC, span] bf16."""
            gam = spool.tile([128, HC, NG], m_dt, tag="gam")
            dom_s = domE_sb[:, wg * SEG_W: wg * SEG_W + span]
            for hc in range(HC):
                pgb = pm.tile([128, H], dt.float32, tag="pmlp")
                nc.tensor.matmul(
                    pgb[:, 0:span],
                    gw_sb[:, hc * 128:(hc + 1) * 128], dom_s,
                    start=True, stop=True)
                nc.scalar.copy(gam[:, hc, 0:span], pgb[:, 0:span])
            gstate[wg // GRP] = (gam, spool.tile(
                [128, HC, NG], m_dt, tag="gmodT", name="gmodT"))

        def emit_mlp(wg, span):
            """MLP for windows [wg, wg+GRP)."""
            gmodT = gstate[wg // GRP][1]
            dom_s = domE_sb[:, wg * SEG_W: wg * SEG_W + span]
            # layer 1 (+ folded beta/beta_b via K=17 matmul)
            h1 = spool.tile([128, HC, NG], m_dt, tag="h1")
            for jc in range(JC):
                ph1 = pm.tile([128, NG], dt.float32, tag="pmlp")
                nc.tensor.matmul(
                    ph1[:, 0:span],
                    w1bw_sb[:, jc * 128:(jc + 1) * 128], dom_s,
                    start=True, stop=False, skip_group_check=True)
                for hc in range(HC):
                    nc.tensor.matmul(
                        ph1[:, 0:span],
                        w1_sb[:, hc, jc * 128:(jc + 1) * 128],
                        gmodT[:, hc, 0:span],
                        start=False, stop=(hc == HC - 1), skip_group_check=True)
                if SILU_COMPOSE:
                    z1 = spool.tile([128, NG], dt.float32, tag="z1")
                    nc.scalar.activation(
                        z1[:, 0:span], ph1[:, 0:span],
                        mybir.ActivationFunctionType.Identity,
                        bias=b1_sb[:, jc:jc + 1])
                    nc.scalar.activation(
                        h1[:, jc, 0:span], z1[:, 0:span],
                        mybir.ActivationFunctionType.Sigmoid)
                    nc.vector.tensor_mul(
                        h1[:, jc, 0:span], h1[:, jc, 0:span], z1[:, 0:span])
                else:
                    nc.scalar.activation(
                        h1[:, jc, 0:span], ph1[:, 0:span],
                        mybir.ActivationFunctionType.Silu,
                        bias=b1_sb[:, jc:jc + 1])
            # layer 2
            h2 = spool.tile([128, KC, NG], m_dt, tag="h2")
            for kc in range(KC):
                ph2 = pm.tile([128, NG], dt.float32, tag="pmlp")
                for hc in range(HC):
                    nc.tensor.matmul(
                        ph2[:, 0:span],
                        w2_sb[:, hc, kc * 128:(kc + 1) * 128],
                        h1[:, hc, 0:span],
                        start=(hc == 0), stop=(hc == HC - 1))
                if SILU_COMPOSE:
                    z2 = spool.tile([128, NG], dt.float32, tag="z2")
                    nc.scalar.activation(
                        z2[:, 0:span], ph2[:, 0:span],
                        mybir.ActivationFunctionType.Identity,
                        bias=b2_sb[:, kc:kc + 1])
                    nc.scalar.activation(
                        h2[:, kc, 0:span], z2[:, 0:span],
                        mybir.ActivationFunctionType.Sigmoid)
                    nc.vector.tensor_mul(
                        h2[:, kc, 0:span], h2[:, kc, 0:span], z2[:, 0:span])
                else:
                    nc.scalar.activation(
                        h2[:, kc, 0:span], ph2[:, 0:span],
                        mybir.ActivationFunctionType.Silu,
                        bias=b2_sb[:, kc:kc + 1])
            # output head
            po = pm.tile([1, NG], dt.float32, tag="pmlp")
            for kc in range(KC):
                nc.tensor.matmul(
                    po[:, 0:span], w3_sb[:, kc:kc + 1], h2[:, kc, 0:span],
                    start=(kc == 0), stop=(kc == KC - 1))
            nc.scalar.activation(
                out_sb[0:1, wg * SEG_W: wg * SEG_W + span], po[:, 0:span],
                mybir.ActivationFunctionType.Identity,
                bias=b3_sb[0:1, 0:1])

        g_sbs = {}

        def emit_transform(w):
            """Transpose g(w) and FiLM-multiply into its group's gmodT."""
            g_sb = g_sbs.pop(w)
            pt_t = pt.tile([128, H], dt.float32, name="pt_t")
            for hc in range(HC):
                nc.tensor.transpose(
                    pt_t[:, hc * 128:(hc + 1) * 128],
                    g_sb[:, hc * 128:(hc + 1) * 128],
                    iden_sb[:])
            wi = w % GRP
            gam, gmodT = gstate[w // GRP]
            pt_v = pt_t[:].rearrange("p (c s) -> p c s", c=HC)
            gm_v = gmodT[:].rearrange("p c (g s) -> p c g s", g=GRP)
            ga_v = gam[:].rearrange("p c (g s) -> p c g s", g=GRP)
            nc.vector.tensor_mul(
                gm_v[:, :, wi, :], pt_v, ga_v[:, :, wi, :])

        mlp_done = set()

        def emit_mlp_group(grp_i):
            if grp_i in mlp_done:
                return
            mlp_done.add(grp_i)
            wg = grp_i * GRP
            emit_mlp(wg, min(NG, spc - wg * SEG_W))

        for w in range(windows):
            if w % GRP == 0:
                emit_gamma(w, min(NG, spc - w * SEG_W))

            # --- segment-sum for this window: accumulate [128 seg, H] ---
            pg_t = pg.tile([128, H], dt.float32)
            off = 0
            for blk in blocks:
                x_sb = xpool.tile([128, XT, H], x_dt)
                nc.sync.dma_start(
                    x_sb[:, 0:blk, :],
                    xp[w][:, off * H:(off + blk) * H]
                    .rearrange("p (c h) -> p c h", c=blk))
                # batched one-hot for the whole block: [128, blk, 128] fp8
                oh = ohpool.tile([128, XT, 128], x_dt)
                iotr_v = iotr_sb[:].rearrange("p (o s) -> p o s", o=1)
                brt_v = brt_sb[:, w * t_tiles + off: w * t_tiles + off + blk]
                brt_v = brt_v.rearrange("p (c o) -> p c o", o=1)
                in0, in1 = bass.broadcast_tensor_aps(iotr_v, brt_v)
                nc.vector.tensor_tensor(oh[:, 0:blk, :], in0, in1, is_eq)
                for gpair in range(blk // 2):
                    ti = off + 2 * gpair
                    nc.tensor.matmul(
                        pg_t[:],
                        oh[:, 2 * gpair:2 * gpair + 2, :],
                        x_sb[:, 2 * gpair:2 * gpair + 2, :],
                        start=(ti == 0), stop=(ti == t_tiles - 2),
                        perf_mode=DR)
                off += blk

            # evict g(w) right behind its stop-matmul (ScalarE overlaps the
            # next window's PE stream; the transpose runs a window later)
            g_sb = spool.tile([128, H], dt.float32, tag="g", name="g_sb")
            nc.scalar.copy(g_sb[:], pg_t[:])
            g_sbs[w] = g_sb

            if w == 0:
                emit_weight_dmas()
            if w >= 1:
                emit_transform(w - 1)
            if w >= 2 and (w - 2) % GRP == GRP - 1:
                emit_mlp_group((w - 2) // GRP)

        emit_transform(windows - 1)
        for grp_i in range((windows + GRP - 1) // GRP):
            emit_mlp_group(grp_i)
        nc.sync.dma_start(out, out_sb[:])

    nc.compile()
    return nc


def _sigma_delta_fp8(x: np.ndarray, batch: np.ndarray, n_segs: int) -> np.ndarray:
    """fp8-e4m3 quantization of x with per-(segment, h) error feedback.

    Within each segment the quantization errors telescope, so segment sums
    of the returned array match the fp32 sums to ~1 quantum.
    """
    starts = np.searchsorted(batch, np.arange(n_segs + 1))
    lens = np.diff(starts)
    L = int(lens.max())
    xq = np.empty(x.shape, dtype=F8)
    order = np.argsort(-lens, kind="stable")  # longest first: shrinking actives
    sorted_lens = lens[order]
    sorted_starts = starts[order]
    carry = np.zeros((n_segs, x.shape[1]), np.float32)
    for k in range(L):
        n_act = int(np.searchsorted(-sorted_lens, -k, side="left"))
        if n_act == 0:
            break
        rows = sorted_starts[:n_act] + k
        v = x[rows] + carry[:n_act]
        q = v.astype(F8)
        carry[:n_act] = v - q.astype(np.float32)
        xq[rows] = q
    return xq


def _balance_windows(batch: np.ndarray, n_segs: int, n_windows: int):
    """Greedy LPT assignment of segments to windows (128 segments each),
    equalizing node counts.  Returns (win_of_seg, slot_of_seg, t_tiles)."""
    counts = np.bincount(batch, minlength=n_segs)
    order = np.argsort(-counts, kind="stable")
    loads = np.zeros(n_windows, np.int64)
    nseg = np.zeros(n_windows, np.int64)
    win = np.empty(n_segs, np.int64)
    INF = 1 << 40
    for s in order:
        eligible = np.where(nseg < SEG_W, loads, INF)
        w = int(np.argmin(eligible))
        win[s] = w
        loads[w] += counts[s]
        nseg[w] += 1
    assert (nseg == SEG_W).all()
    # slot of each segment within its window (stable by segment id)
    o = np.argsort(win, kind="stable")
    slot = np.empty(n_segs, np.int64)
    slot[o] = np.arange(n_segs) - np.repeat(
        np.arange(n_windows) * SEG_W, SEG_W)
    t_tiles = max(2, 2 * int(-(-loads.max() // 256)))
    return win, slot, t_tiles


def prepare_core_inputs(
    x, batch, domain_emb, gamma_w, gamma_b, beta_w, beta_b,
    w1, b1, w2, b2, w3, b3,
    spc: int, n_cores: int, plan=None,
):
    """Slice/pad/pack the full inputs into one in_map per core.

    Returns (in_maps, seg_pos) where seg_pos[seg] is the segment's position
    in the permuted, concatenated output."""
    n_segs = spc * n_cores
    windows = spc // SEG_W
    n_win_tot = windows * n_cores

    batch = np.ascontiguousarray(np.asarray(batch).astype(np.int64))
    x = np.asarray(x, dtype=np.float32)

    if plan is None:
        plan = _balance_windows(batch, n_segs, n_win_tot)
    win, slot, t_tiles = plan
    npw = SEG_W * t_tiles

    w1_f = np.asarray(w1, np.float32)
    bw_ext = np.concatenate([np.asarray(beta_w, np.float32).T,
                             np.asarray(beta_b, np.float32)[None]], axis=0)  # [17, H]
    w1bw = bw_ext @ w1_f.T                                                   # [17, H]

    shared = {
        "gw": np.ascontiguousarray(_f32_to_bf16(
            np.concatenate([np.asarray(gamma_w, np.float32).T,
                            np.asarray(gamma_b, np.float32)[None]], axis=0))),
        "w1bw": np.ascontiguousarray(_f32_to_bf16(w1bw)),
        "w1t": np.ascontiguousarray(_f32_to_bf16(w1_f.T)),
        "w2t": np.ascontiguousarray(_f32_to_bf16(np.asarray(w2, np.float32).T)),
        "w3c": np.ascontiguousarray(
            _f32_to_bf16(np.asarray(w3, np.float32).reshape(H2 // 128, 128).T)),
        "b1c": np.ascontiguousarray(np.asarray(b1, np.float32).reshape(H // 128, 128).T),
        "b2c": np.ascontiguousarray(np.asarray(b2, np.float32).reshape(H2 // 128, 128).T),
        "b3c": np.asarray(b3, np.float32).reshape(1, 1),
        "iden": np.eye(128, dtype=np.float32),
        "iotr": np.tile(np.arange(128, dtype=np.float32), (128, 1)).astype(BF16),
    }

    xq_u8 = _sigma_delta_fp8(x, batch, n_segs).view(np.uint8)

    # permuted node order: grouped by window (stable, so per-segment runs stay
    # contiguous), with per-node window/slot ids
    node_win = win[batch]
    node_slot = slot[batch].astype(np.float32)
    order = np.argsort(node_win, kind="stable")
    wstarts = np.searchsorted(node_win[order], np.arange(n_win_tot + 1))

    # segment position in the permuted output
    seg_pos = win * SEG_W + slot

    dom = np.asarray(domain_emb, np.float32)
    dom_ext = np.concatenate([dom.T, np.ones((1, n_segs), np.float32)], axis=0)
    domP = np.empty((FD + 1, n_segs), np.float32)
    domP[:, seg_pos] = dom_ext

    in_maps = []
    for core in range(n_cores):
        xp_c = np.zeros((windows, npw, H), dtype=np.uint8)
        brt_c = np.full((windows, npw), -1024.0, dtype=np.float32)
        for wl in range(windows):
            wg = core * windows + wl
            ns = order[wstarts[wg]:wstarts[wg + 1]]
            cnt = len(ns)
            if cnt > npw:
                raise ValueError(f"window overflow: {cnt} > {npw}")
            if cnt == 0:
                continue
            xp_c[wl, :cnt] = xq_u8[ns]
            brt_c[wl, :cnt] = node_slot[ns]
        # [windows, npw, H] -> [windows, 128, t_tiles*H]: node c*128+p at
        # partition p, free slot (c, h)
        xp_c = np.ascontiguousarray(
            xp_c.reshape(windows, t_tiles, 128, H)
            .transpose(0, 2, 1, 3)
            .reshape(windows, 128, t_tiles * H)).view(F8)
        # [windows, npw] -> [128, windows*t_tiles]: brt[p, w*t_tiles+ti]
        brt_c = np.ascontiguousarray(
            brt_c.reshape(windows, t_tiles, 128).transpose(2, 0, 1)
            .reshape(128, windows * t_tiles).astype(BF16))
        domE_c = np.ascontiguousarray(
            _f32_to_bf16(domP[:, core * spc:(core + 1) * spc]))
        in_maps.append({"xp": xp_c, "brt": brt_c, "domE": domE_c, **shared})
    return in_maps, seg_pos, t_tiles


_PROGRAM_CACHE: dict = {}

# Set by test harnesses: request an NTFF trace and stash the raw results.
TRACE = False
LAST_RESULT = None


def kernel(**inputs) -> np.ndarray:
    x = np.asarray(inputs["x"], dtype=np.float32)
    batch = np.ascontiguousarray(np.asarray(inputs["batch"]).astype(np.int64))
    assert x.shape == (N_NODES, H), x.shape

    spc = B_SEGS // N_CORES

    in_maps, seg_pos, t_tiles = prepare_core_inputs(
        x, batch,
        inputs["domain_emb"], inputs["gamma_w"], inputs["gamma_b"],
        inputs["beta_w"], inputs["beta_b"],
        inputs["w1"], inputs["b1"], inputs["w2"], inputs["b2"],
        inputs["w3"], inputs["b3"],
        spc, N_CORES,
    )

    key = (spc, t_tiles, N_CORES)
    if key not in _PROGRAM_CACHE:
        _PROGRAM_CACHE[key] = build_program(spc, t_tiles, N_CORES)
    nc = _PROGRAM_CACHE[key]

    res = bass_utils.run_bass_kernel_spmd(
        nc, in_maps, core_ids=list(range(N_CORES)), trace=TRACE)
    global LAST_RESULT
    LAST_RESULT = res
    out_perm = np.concatenate(
        [res.results[c]["out"].reshape(-1) for c in range(N_CORES)])
    return np.ascontiguousarray(out_perm[seg_pos].astype(np.float32))


# revision 13
# speedup vs baseline: 2.4609x; 1.0294x over previous
"""Trainium2 Bass kernel for nn_GemNetOutput (segment_reduce + FiLM + MLP head).

Reference computation (all fp32):
    g     = segment_sum(x, batch, num_segments=B)        # [B, H]
    gamma = domain_emb @ gamma_w.T + gamma_b             # [B, H]
    beta  = domain_emb @ beta_w.T  + beta_b              # [B, H]
    g     = gamma * g + beta
    h     = silu(g @ w1.T + b1)                          # [B, H]
    h     = silu(h @ w2.T + b2)                          # [B, H/2]
    out   = (h @ w3.T + b3).squeeze(-1)                  # [B]

Shapes: N=1e6 nodes, B=16384 graphs, H=512, FD=16.  `batch` is SORTED.

Strategy (8 NeuronCores, no collectives needed):
  - The 16384 segments are BIN-PACKED into 128 windows of exactly 128
    segments each, equalizing the node count per window (greedy LPT on the
    host).  Core c owns windows [c*16, (c+1)*16).  All cores run one
    identical static program; every window is padded to the same t_tiles
    node tiles (sentinel one-hot ids mask the padding) -- balancing makes
    that padding ~1.5% instead of ~5%.
  - x is converted to fp8-e4m3 on the host with SIGMA-DELTA (error-feedback)
    rounding along each segment: quantization errors telescope within a
    segment, so the device's segment sums match the fp32 sums to ~1 quantum
    instead of sqrt(n) quanta.  Halves HBM traffic vs bf16.
  - x is packed on the host into [window, 128, t_tiles*H] so each DMA block
    is a fully contiguous ~1 MB transfer (8 KB per partition line).
  - segment_sum on the PE with fp8 DoubleRow matmuls: each matmul consumes
    TWO 128-node tiles (contraction 256) against a [128, 2, 128] one-hot.
  - One-hot built on DVE with one batched tensor_tensor per DMA block
    (stride-0 broadcast APs) instead of one tensor_scalar per tile.
  - beta (incl. beta_b) is folded into the MLP-1 accumulation on the host:
    ph1 += (W1 @ beta_w_ext.T) @ dom_ext, one small K=17 matmul per j-chunk.
  - FiLM multiply + MLP run per GROUP of 4 windows in transposed
    [feature, seg] layout so MLP matmuls have N=512 moving operands.
"""

import sys
from contextlib import ExitStack

for _p in ("/opt/trn_rl_repo", "/opt/pypackages"):
    if _p not in sys.path:
        sys.path.append(_p)

import ml_dtypes
import numpy as np

import concourse.bass as bass
import concourse.tile as tile
from concourse import bacc, mybir
from concourse import bass_utils

dt = mybir.dt

# Problem constants (hardcoded per the contract).
N_NODES = 1_000_000
B_SEGS = 16_384
H = 512
H2 = 256
FD = 16
N_CORES = 8
SEG_W = 128          # segments per window (PSUM partition dim)
GRP = 4              # windows per MLP group (moving N = GRP*SEG_W = 512)
XT = 16              # max node subtiles (of 128 rows) per x DMA block

BF16 = ml_dtypes.bfloat16
F8 = ml_dtypes.float8_e4m3fn

# CoreSim has no Silu LUT; compose silu = z * sigmoid(z) when True (sim tests).
SILU_COMPOSE = False


def _f32_to_bf16(a: np.ndarray) -> np.ndarray:
    return np.ascontiguousarray(a, dtype=np.float32).astype(BF16)


def _blocks_of(t_tiles: int) -> tuple:
    """Split t_tiles into even-sized DMA blocks of at most XT subtiles."""
    blocks = [XT] * (t_tiles // XT)
    rem = t_tiles % XT
    if rem:
        blocks.append(rem)
    assert all(b % 2 == 0 for b in blocks)
    return tuple(blocks)


def build_program(spc: int, t_tiles: int, n_cores: int):
    """Build the per-core Bass/Tile program.

    spc: segments per core (multiple of 128)
    t_tiles: node tiles (of 128) per 128-segment window; even
    """
    assert t_tiles % 2 == 0
    windows = spc // SEG_W
    blocks = _blocks_of(t_tiles)
    x_dt = dt.float8e4
    m_dt = dt.bfloat16             # MLP matmul dtype

    nc = bacc.Bacc(
        "TRN2",
        target_bir_lowering=False,
        debug=False,
        enable_asserts=False,
        num_devices=n_cores,
    )

    xp = nc.dram_tensor("xp", [windows, 128, t_tiles * H], x_dt,
                        kind="ExternalInput").ap()
    brt = nc.dram_tensor("brt", [128, windows * t_tiles], dt.bfloat16,
                         kind="ExternalInput").ap()
    domE = nc.dram_tensor("domE", [FD + 1, spc], m_dt, kind="ExternalInput").ap()
    gw = nc.dram_tensor("gw", [FD + 1, H], m_dt, kind="ExternalInput").ap()
    w1bw = nc.dram_tensor("w1bw", [FD + 1, H], m_dt, kind="ExternalInput").ap()
    w1t = nc.dram_tensor("w1t", [H, H], m_dt, kind="ExternalInput").ap()
    w2t = nc.dram_tensor("w2t", [H, H2], m_dt, kind="ExternalInput").ap()
    w3c = nc.dram_tensor("w3c", [128, H2 // 128], m_dt, kind="ExternalInput").ap()
    b1c = nc.dram_tensor("b1c", [128, H // 128], dt.float32, kind="ExternalInput").ap()
    b2c = nc.dram_tensor("b2c", [128, H2 // 128], dt.float32, kind="ExternalInput").ap()
    b3c = nc.dram_tensor("b3c", [1, 1], dt.float32, kind="ExternalInput").ap()
    iden = nc.dram_tensor("iden", [128, 128], dt.float32, kind="ExternalInput").ap()
    iotr = nc.dram_tensor("iotr", [128, 128], dt.bfloat16, kind="ExternalInput").ap()
    out = nc.dram_tensor("out", [1, spc], dt.float32, kind="ExternalOutput").ap()

    HC = H // 128       # 4 h-chunks
    JC = H // 128       # 4 layer-1 output chunks
    KC = H2 // 128      # 2 layer-2 output chunks
    NG = GRP * SEG_W    # max moving width of group-level MLP matmuls

    # group sizes: 4-window groups, but finish with 2/1/1 so the final MLP
    # chains are short and overlap the x DMA stream instead of trailing it
    gplan = []
    rem = windows
    while rem > 4:
        gplan.append(4)
        rem -= 4
    gplan.extend([2, 1, 1] if rem == 4 else [1] * rem)
    gstarts = [sum(gplan[:i]) for i in range(len(gplan))]
    group_of = {}
    for gi, (wg, gsz) in enumerate(zip(gstarts, gplan)):
        for w in range(wg, wg + gsz):
            group_of[w] = gi

    is_eq = mybir.AluOpType.is_equal
    DR = mybir.MatmulPerfMode.DoubleRow

    with tile.TileContext(nc) as tc, ExitStack() as ctx:
        cpool = ctx.enter_context(tc.tile_pool(name="consts", bufs=1))
        xpool = ctx.enter_context(tc.tile_pool(name="x", bufs=8))
        ohpool = ctx.enter_context(tc.tile_pool(name="oh", bufs=6))
        spool = ctx.enter_context(tc.tile_pool(name="work", bufs=2))
        pg = ctx.enter_context(tc.tile_pool(name="pg", bufs=2, space=bass.MemorySpace.PSUM))
        pt = ctx.enter_context(tc.tile_pool(name="pt", bufs=2, space=bass.MemorySpace.PSUM))
        pm = ctx.enter_context(tc.tile_pool(name="pm", bufs=3, space=bass.MemorySpace.PSUM))

        # ---- constants / weights into SBUF ----
        iden_sb = cpool.tile([128, 128], dt.float32)
        nc.sync.dma_start(iden_sb[:], iden)
        iotr_sb = cpool.tile([128, 128], dt.bfloat16)
        nc.sync.dma_start(iotr_sb[:], iotr)
        w1_sb = cpool.tile([128, HC, H], m_dt)
        w2_sb = cpool.tile([128, HC, H2], m_dt)
        w3_sb = cpool.tile([128, KC], m_dt)
        b1_sb = cpool.tile([128, JC], dt.float32)
        b2_sb = cpool.tile([128, KC], dt.float32)
        b3_sb = cpool.tile([1, 1], dt.float32)

        def emit_weight_dmas():
            nc.sync.dma_start(w1_sb[:], w1t.rearrange("(c p) j -> p c j", p=128))
            nc.sync.dma_start(w2_sb[:], w2t.rearrange("(c p) j -> p c j", p=128))
            nc.sync.dma_start(w3_sb[:], w3c)
            nc.sync.dma_start(b1_sb[:], b1c)
            nc.sync.dma_start(b2_sb[:], b2c)
            nc.sync.dma_start(b3_sb[:], b3c)
        gw_sb = cpool.tile([FD + 1, H], m_dt)
        nc.sync.dma_start(gw_sb[:], gw)
        w1bw_sb = cpool.tile([FD + 1, H], m_dt)
        nc.sync.dma_start(w1bw_sb[:], w1bw)
        domE_sb = cpool.tile([FD + 1, spc], m_dt)
        nc.sync.dma_start(domE_sb[:], domE)
        brt_sb = cpool.tile([128, windows * t_tiles], dt.bfloat16)
        nc.sync.dma_start(brt_sb[:], brt)
        out_sb = cpool.tile([1, spc], dt.float32)

        # ---- PE warm-up: ~4.5us of dummy matmuls while DMA prefills, so HAM
        # flips to K=8/8 before the real stream starts.
        warm_t = pm.tile([128, H], dt.float32, tag="pmlp")
        for i in range(44):
            nc.tensor.matmul(
                warm_t[:, 0:128], iotr_sb[:], iotr_sb[:],
                start=(i == 0), stop=(i == 43))

        gstate = {}

        def emit_gamma(gi):
            """gammaT for group gi: [128 h, HC, span] bf16."""
            wg, span = gstarts[gi], gplan[gi] * SEG_W
            gam = spool.tile([128, HC, NG], m_dt, tag="gam")
            dom_s = domE_sb[:, wg * SEG_W: wg * SEG_W + span]
            for hc in range(HC):
                pgb = pm.tile([128, H], dt.float32, tag="pmlp")
                nc.tensor.matmul(
                    pgb[:, 0:span],
                    gw_sb[:, hc * 128:(hc + 1) * 128], dom_s,
                    start=True, stop=True)
                nc.scalar.copy(gam[:, hc, 0:span], pgb[:, 0:span])
            gstate[gi] = (gam, spool.tile(
                [128, HC, NG], m_dt, tag="gmodT", name="gmodT"))

        def emit_mlp(wg, span, gi):
            """MLP for group gi = windows [wg, wg+span/SEG_W)."""
            gmodT = gstate[gi][1]
            dom_s = domE_sb[:, wg * SEG_W: wg * SEG_W + span]
            # layer 1 (+ folded beta/beta_b via K=17 matmul)
            h1 = spool.tile([128, HC, NG], m_dt, tag="h1")
            for jc in range(JC):
                ph1 = pm.tile([128, NG], dt.float32, tag="pmlp")
                nc.tensor.matmul(
                    ph1[:, 0:span],
                    w1bw_sb[:, jc * 128:(jc + 1) * 128], dom_s,
                    start=True, stop=False, skip_group_check=True)
                for hc in range(HC):
                    nc.tensor.matmul(
                        ph1[:, 0:span],
                        w1_sb[:, hc, jc * 128:(jc + 1) * 128],
                        gmodT[:, hc, 0:span],
                        start=False, stop=(hc == HC - 1), skip_group_check=True)
                if SILU_COMPOSE:
                    z1 = spool.tile([128, NG], dt.float32, tag="z1")
                    nc.scalar.activation(
                        z1[:, 0:span], ph1[:, 0:span],
                        mybir.ActivationFunctionType.Identity,
                        bias=b1_sb[:, jc:jc + 1])
                    nc.scalar.activation(
                        h1[:, jc, 0:span], z1[:, 0:span],
                        mybir.ActivationFunctionType.Sigmoid)
                    nc.vector.tensor_mul(
                        h1[:, jc, 0:span], h1[:, jc, 0:span], z1[:, 0:span])
                else:
                    nc.scalar.activation(
                        h1[:, jc, 0:span], ph1[:, 0:span],
                        mybir.ActivationFunctionType.Silu,
                        bias=b1_sb[:, jc:jc + 1])
            # layer 2
            h2 = spool.tile([128, KC, NG], m_dt, tag="h2")
            for kc in range(KC):
                ph2 = pm.tile([128, NG], dt.float32, tag="pmlp")
                for hc in range(HC):
                    nc.tensor.matmul(
                        ph2[:, 0:span],
                        w2_sb[:, hc, kc * 128:(kc + 1) * 128],
                        h1[:, hc, 0:span],
                        start=(hc == 0), stop=(hc == HC - 1))
                if SILU_COMPOSE:
                    z2 = spool.tile([128, NG], dt.float32, tag="z2")
                    nc.scalar.activation(
                        z2[:, 0:span], ph2[:, 0:span],
                        mybir.ActivationFunctionType.Identity,
                        bias=b2_sb[:, kc:kc + 1])
                    nc.scalar.activation(
                        h2[:, kc, 0:span], z2[:, 0:span],
                        mybir.ActivationFunctionType.Sigmoid)
                    nc.vector.tensor_mul(
                        h2[:, kc, 0:span], h2[:, kc, 0:span], z2[:, 0:span])
                else:
                    nc.scalar.activation(
                        h2[:, kc, 0:span], ph2[:, 0:span],
                        mybir.ActivationFunctionType.Silu,
                        bias=b2_sb[:, kc:kc + 1])
            # output head
            po = pm.tile([1, NG], dt.float32, tag="pmlp")
            for kc in range(KC):
                nc.tensor.matmul(
                    po[:, 0:span], w3_sb[:, kc:kc + 1], h2[:, kc, 0:span],
                    start=(kc == 0), stop=(kc == KC - 1))
            nc.scalar.activation(
                out_sb[0:1, wg * SEG_W: wg * SEG_W + span], po[:, 0:span],
                mybir.ActivationFunctionType.Identity,
                bias=b3_sb[0:1, 0:1])
            nc.sync.dma_start(
                out[0:1, wg * SEG_W: wg * SEG_W + span],
                out_sb[0:1, wg * SEG_W: wg * SEG_W + span])

        g_sbs = {}

        def emit_transform(w):
            """Transpose g(w) and FiLM-multiply into its group's gmodT."""
            g_sb = g_sbs.pop(w)
            pt_t = pt.tile([128, H], dt.float32, name="pt_t")
            for hc in range(HC):
                nc.tensor.transpose(
                    pt_t[:, hc * 128:(hc + 1) * 128],
                    g_sb[:, hc * 128:(hc + 1) * 128],
                    iden_sb[:])
            gi = group_of[w]
            wi = w - gstarts[gi]
            gam, gmodT = gstate[gi]
            pt_v = pt_t[:].rearrange("p (c s) -> p c s", c=HC)
            gm_v = gmodT[:].rearrange("p c (g s) -> p c g s", g=GRP)
            ga_v = gam[:].rearrange("p c (g s) -> p c g s", g=GRP)
            nc.vector.tensor_mul(
                gm_v[:, :, wi, :], pt_v, ga_v[:, :, wi, :])

        mlp_done = set()

        def emit_mlp_group(grp_i):
            if grp_i in mlp_done:
                return
            mlp_done.add(grp_i)
            emit_mlp(gstarts[grp_i], gplan[grp_i] * SEG_W, grp_i)

        for w in range(windows):
            if w in gstarts:
                emit_gamma(group_of[w])

            # --- segment-sum for this window: accumulate [128 seg, H] ---
            pg_t = pg.tile([128, H], dt.float32)
            off = 0
            for blk in blocks:
                x_sb = xpool.tile([128, XT, H], x_dt)
                nc.sync.dma_start(
                    x_sb[:, 0:blk, :],
                    xp[w][:, off * H:(off + blk) * H]
                    .rearrange("p (c h) -> p c h", c=blk))
                # batched one-hot for the whole block: [128, blk, 128] fp8
                oh = ohpool.tile([128, XT, 128], x_dt)
                iotr_v = iotr_sb[:].rearrange("p (o s) -> p o s", o=1)
                brt_v = brt_sb[:, w * t_tiles + off: w * t_tiles + off + blk]
                brt_v = brt_v.rearrange("p (c o) -> p c o", o=1)
                in0, in1 = bass.broadcast_tensor_aps(iotr_v, brt_v)
                nc.vector.tensor_tensor(oh[:, 0:blk, :], in0, in1, is_eq)
                for gpair in range(blk // 2):
                    ti = off + 2 * gpair
                    nc.tensor.matmul(
                        pg_t[:],
                        oh[:, 2 * gpair:2 * gpair + 2, :],
                        x_sb[:, 2 * gpair:2 * gpair + 2, :],
                        start=(ti == 0), stop=(ti == t_tiles - 2),
                        perf_mode=DR)
                off += blk

            # evict g(w) right behind its stop-matmul (ScalarE overlaps the
            # next window's PE stream; the transpose runs a window later)
            g_sb = spool.tile([128, H], dt.float32, tag="g", name="g_sb")
            nc.scalar.copy(g_sb[:], pg_t[:])
            g_sbs[w] = g_sb

            if w == 0:
                emit_weight_dmas()
            if w >= 1:
                emit_transform(w - 1)
            if w >= 2:
                g2 = group_of[w - 2]
                if w - 2 == gstarts[g2] + gplan[g2] - 1:
                    emit_mlp_group(g2)

        emit_transform(windows - 1)
        for grp_i in range(len(gplan)):
            emit_mlp_group(grp_i)

    nc.compile()
    return nc


def _sigma_delta_fp8(x: np.ndarray, batch: np.ndarray, n_segs: int) -> np.ndarray:
    """fp8-e4m3 quantization of x with per-(segment, h) error feedback.

    Within each segment the quantization errors telescope, so segment sums
    of the returned array match the fp32 sums to ~1 quantum.
    """
    starts = np.searchsorted(batch, np.arange(n_segs + 1))
    lens = np.diff(starts)
    L = int(lens.max())
    xq = np.empty(x.shape, dtype=F8)
    order = np.argsort(-lens, kind="stable")  # longest first: shrinking actives
    sorted_lens = lens[order]
    sorted_starts = starts[order]
    carry = np.zeros((n_segs, x.shape[1]), np.float32)
    for k in range(L):
        n_act = int(np.searchsorted(-sorted_lens, -k, side="left"))
        if n_act == 0:
            break
        rows = sorted_starts[:n_act] + k
        v = x[rows] + carry[:n_act]
        q = v.astype(F8)
        carry[:n_act] = v - q.astype(np.float32)
        xq[rows] = q
    return xq


def _balance_windows(batch: np.ndarray, n_segs: int, n_windows: int):
    """Greedy LPT assignment of segments to windows (128 segments each),
    equalizing node counts.  Returns (win_of_seg, slot_of_seg, t_tiles)."""
    counts = np.bincount(batch, minlength=n_segs)
    order = np.argsort(-counts, kind="stable")
    loads = np.zeros(n_windows, np.int64)
    nseg = np.zeros(n_windows, np.int64)
    win = np.empty(n_segs, np.int64)
    INF = 1 << 40
    for s in order:
        eligible = np.where(nseg < SEG_W, loads, INF)
        w = int(np.argmin(eligible))
        win[s] = w
        loads[w] += counts[s]
        nseg[w] += 1
    assert (nseg == SEG_W).all()
    # slot of each segment within its window (stable by segment id)
    o = np.argsort(win, kind="stable")
    slot = np.empty(n_segs, np.int64)
    slot[o] = np.arange(n_segs) - np.repeat(
        np.arange(n_windows) * SEG_W, SEG_W)
    t_tiles = max(2, 2 * int(-(-loads.max() // 256)))
    return win, slot, t_tiles


def prepare_core_inputs(
    x, batch, domain_emb, gamma_w, gamma_b, beta_w, beta_b,
    w1, b1, w2, b2, w3, b3,
    spc: int, n_cores: int, plan=None,
):
    """Slice/pad/pack the full inputs into one in_map per core.

    Returns (in_maps, seg_pos) where seg_pos[seg] is the segment's position
    in the permuted, concatenated output."""
    n_segs = spc * n_cores
    windows = spc // SEG_W
    n_win_tot = windows * n_cores

    batch = np.ascontiguousarray(np.asarray(batch).astype(np.int64))
    x = np.asarray(x, dtype=np.float32)

    if plan is None:
        plan = _balance_windows(batch, n_segs, n_win_tot)
    win, slot, t_tiles = plan
    npw = SEG_W * t_tiles

    w1_f = np.asarray(w1, np.float32)
    bw_ext = np.concatenate([np.asarray(beta_w, np.float32).T,
                             np.asarray(beta_b, np.float32)[None]], axis=0)  # [17, H]
    w1bw = bw_ext @ w1_f.T                                                   # [17, H]

    shared = {
        "gw": np.ascontiguousarray(_f32_to_bf16(
            np.concatenate([np.asarray(gamma_w, np.float32).T,
                            np.asarray(gamma_b, np.float32)[None]], axis=0))),
        "w1bw": np.ascontiguousarray(_f32_to_bf16(w1bw)),
        "w1t": np.ascontiguousarray(_f32_to_bf16(w1_f.T)),
        "w2t": np.ascontiguousarray(_f32_to_bf16(np.asarray(w2, np.float32).T)),
        "w3c": np.ascontiguousarray(
            _f32_to_bf16(np.asarray(w3, np.float32).reshape(H2 // 128, 128).T)),
        "b1c": np.ascontiguousarray(np.asarray(b1, np.float32).reshape(H // 128, 128).T),
        "b2c": np.ascontiguousarray(np.asarray(b2, np.float32).reshape(H2 // 128, 128).T),
        "b3c": np.asarray(b3, np.float32).reshape(1, 1),
        "iden": np.eye(128, dtype=np.float32),
        "iotr": np.tile(np.arange(128, dtype=np.float32), (128, 1)).astype(BF16),
    }

    xq_u8 = _sigma_delta_fp8(x, batch, n_segs).view(np.uint8)

    # permuted node order: grouped by window (stable, so per-segment runs stay
    # contiguous), with per-node window/slot ids
    node_win = win[batch]
    node_slot = slot[batch].astype(np.float32)
    order = np.argsort(node_win, kind="stable")
    wstarts = np.searchsorted(node_win[order], np.arange(n_win_tot + 1))

    # segment position in the permuted output
    seg_pos = win * SEG_W + slot

    dom = np.asarray(domain_emb, np.float32)
    dom_ext = np.concatenate([dom.T, np.ones((1, n_segs), np.float32)], axis=0)
    domP = np.empty((FD + 1, n_segs), np.float32)
    domP[:, seg_pos] = dom_ext

    in_maps = []
    for core in range(n_cores):
        xp_c = np.zeros((windows, npw, H), dtype=np.uint8)
        brt_c = np.full((windows, npw), -1024.0, dtype=np.float32)
        for wl in range(windows):
            wg = core * windows + wl
            ns = order[wstarts[wg]:wstarts[wg + 1]]
            cnt = len(ns)
            if cnt > npw:
                raise ValueError(f"window overflow: {cnt} > {npw}")
            if cnt == 0:
                continue
            xp_c[wl, :cnt] = xq_u8[ns]
            brt_c[wl, :cnt] = node_slot[ns]
        # [windows, npw, H] -> [windows, 128, t_tiles*H]: node c*128+p at
        # partition p, free slot (c, h)
        xp_c = np.ascontiguousarray(
            xp_c.reshape(windows, t_tiles, 128, H)
            .transpose(0, 2, 1, 3)
            .reshape(windows, 128, t_tiles * H)).view(F8)
        # [windows, npw] -> [128, windows*t_tiles]: brt[p, w*t_tiles+ti]
        brt_c = np.ascontiguousarray(
            brt_c.reshape(windows, t_tiles, 128).transpose(2, 0, 1)
            .reshape(128, windows * t_tiles).astype(BF16))
        domE_c = np.ascontiguousarray(
            _f32_to_bf16(domP[:, core * spc:(core + 1) * spc]))
        in_maps.append({"xp": xp_c, "brt": brt_c, "domE": domE_c, **shared})
    return in_maps, seg_pos, t_tiles


_PROGRAM_CACHE: dict = {}

# Set by test harnesses: request an NTFF trace and stash the raw results.
TRACE = False
LAST_RESULT = None


def kernel(**inputs) -> np.ndarray:
    x = np.asarray(inputs["x"], dtype=np.float32)
    batch = np.ascontiguousarray(np.asarray(inputs["batch"]).astype(np.int64))
    assert x.shape == (N_NODES, H), x.shape

    spc = B_SEGS // N_CORES

    in_maps, seg_pos, t_tiles = prepare_core_inputs(
        x, batch,
        inputs["domain_emb"], inputs["gamma_w"], inputs["gamma_b"],
        inputs["beta_w"], inputs["beta_b"],
        inputs["w1"], inputs["b1"], inputs["w2"], inputs["b2"],
        inputs["w3"], inputs["b3"],
        spc, N_CORES,
    )

    key = (spc, t_tiles, N_CORES)
    if key not in _PROGRAM_CACHE:
        _PROGRAM_CACHE[key] = build_program(spc, t_tiles, N_CORES)
    nc = _PROGRAM_CACHE[key]

    res = bass_utils.run_bass_kernel_spmd(
        nc, in_maps, core_ids=list(range(N_CORES)), trace=TRACE)
    global LAST_RESULT
    LAST_RESULT = res
    out_perm = np.concatenate(
        [res.results[c]["out"].reshape(-1) for c in range(N_CORES)])
    return np.ascontiguousarray(out_perm[seg_pos].astype(np.float32))


# revision 14
# speedup vs baseline: 2.4684x; 1.0031x over previous
"""Trainium2 Bass kernel for nn_GemNetOutput (segment_reduce + FiLM + MLP head).

Reference computation (all fp32):
    g     = segment_sum(x, batch, num_segments=B)        # [B, H]
    gamma = domain_emb @ gamma_w.T + gamma_b             # [B, H]
    beta  = domain_emb @ beta_w.T  + beta_b              # [B, H]
    g     = gamma * g + beta
    h     = silu(g @ w1.T + b1)                          # [B, H]
    h     = silu(h @ w2.T + b2)                          # [B, H/2]
    out   = (h @ w3.T + b3).squeeze(-1)                  # [B]

Shapes: N=1e6 nodes, B=16384 graphs, H=512, FD=16.  `batch` is SORTED.

Strategy (8 NeuronCores, no collectives needed):
  - The 16384 segments are BIN-PACKED into 128 windows of exactly 128
    segments each, equalizing the node count per window (greedy LPT on the
    host).  Core c owns windows [c*16, (c+1)*16).  All cores run one
    identical static program; every window is padded to the same t_tiles
    node tiles (sentinel one-hot ids mask the padding) -- balancing makes
    that padding ~1.5% instead of ~5%.
  - x is converted to fp8-e4m3 on the host with SIGMA-DELTA (error-feedback)
    rounding along each segment: quantization errors telescope within a
    segment, so the device's segment sums match the fp32 sums to ~1 quantum
    instead of sqrt(n) quanta.  Halves HBM traffic vs bf16.
  - x is packed on the host into [window, 128, t_tiles*H] so each DMA block
    is a fully contiguous ~1 MB transfer (8 KB per partition line).
  - segment_sum on the PE with fp8 DoubleRow matmuls: each matmul consumes
    TWO 128-node tiles (contraction 256) against a [128, 2, 128] one-hot.
  - One-hot built on DVE with one batched tensor_tensor per DMA block
    (stride-0 broadcast APs) instead of one tensor_scalar per tile.
  - beta (incl. beta_b) is folded into the MLP-1 accumulation on the host:
    ph1 += (W1 @ beta_w_ext.T) @ dom_ext, one small K=17 matmul per j-chunk.
  - FiLM multiply + MLP run per GROUP of 4 windows in transposed
    [feature, seg] layout so MLP matmuls have N=512 moving operands.
"""

import sys
from contextlib import ExitStack

for _p in ("/opt/trn_rl_repo", "/opt/pypackages"):
    if _p not in sys.path:
        sys.path.append(_p)

import ml_dtypes
import numpy as np

import concourse.bass as bass
import concourse.tile as tile
from concourse import bacc, mybir
from concourse import bass_utils

dt = mybir.dt

# Problem constants (hardcoded per the contract).
N_NODES = 1_000_000
B_SEGS = 16_384
H = 512
H2 = 256
FD = 16
N_CORES = 8
SEG_W = 128          # segments per window (PSUM partition dim)
GRP = 4              # windows per MLP group (moving N = GRP*SEG_W = 512)
XT = 32              # max node subtiles (of 128 rows) per x DMA block

BF16 = ml_dtypes.bfloat16
F8 = ml_dtypes.float8_e4m3fn

# CoreSim has no Silu LUT; compose silu = z * sigmoid(z) when True (sim tests).
SILU_COMPOSE = False


def _f32_to_bf16(a: np.ndarray) -> np.ndarray:
    return np.ascontiguousarray(a, dtype=np.float32).astype(BF16)


def _blocks_of(t_tiles: int) -> tuple:
    """Split t_tiles into even-sized DMA blocks of at most XT subtiles."""
    blocks = [XT] * (t_tiles // XT)
    rem = t_tiles % XT
    if rem:
        blocks.append(rem)
    assert all(b % 2 == 0 for b in blocks)
    return tuple(blocks)


def build_program(spc: int, t_tiles: int, n_cores: int):
    """Build the per-core Bass/Tile program.

    spc: segments per core (multiple of 128)
    t_tiles: node tiles (of 128) per 128-segment window; even
    """
    assert t_tiles % 2 == 0
    windows = spc // SEG_W
    blocks = _blocks_of(t_tiles)
    x_dt = dt.float8e4
    m_dt = dt.bfloat16             # MLP matmul dtype

    nc = bacc.Bacc(
        "TRN2",
        target_bir_lowering=False,
        debug=False,
        enable_asserts=False,
        num_devices=n_cores,
    )

    xp = nc.dram_tensor("xp", [windows, 128, t_tiles * H], x_dt,
                        kind="ExternalInput").ap()
    brt = nc.dram_tensor("brt", [128, windows * t_tiles], dt.bfloat16,
                         kind="ExternalInput").ap()
    domE = nc.dram_tensor("domE", [FD + 1, spc], m_dt, kind="ExternalInput").ap()
    gw = nc.dram_tensor("gw", [FD + 1, H], m_dt, kind="ExternalInput").ap()
    w1bw = nc.dram_tensor("w1bw", [FD + 1, H], m_dt, kind="ExternalInput").ap()
    w1t = nc.dram_tensor("w1t", [H, H], m_dt, kind="ExternalInput").ap()
    w2t = nc.dram_tensor("w2t", [H, H2], m_dt, kind="ExternalInput").ap()
    w3c = nc.dram_tensor("w3c", [128, H2 // 128], m_dt, kind="ExternalInput").ap()
    b1c = nc.dram_tensor("b1c", [128, H // 128], dt.float32, kind="ExternalInput").ap()
    b2c = nc.dram_tensor("b2c", [128, H2 // 128], dt.float32, kind="ExternalInput").ap()
    b3c = nc.dram_tensor("b3c", [1, 1], dt.float32, kind="ExternalInput").ap()
    iden = nc.dram_tensor("iden", [128, 128], dt.float32, kind="ExternalInput").ap()
    iotr = nc.dram_tensor("iotr", [128, 128], dt.bfloat16, kind="ExternalInput").ap()
    out = nc.dram_tensor("out", [1, spc], dt.float32, kind="ExternalOutput").ap()

    HC = H // 128       # 4 h-chunks
    JC = H // 128       # 4 layer-1 output chunks
    KC = H2 // 128      # 2 layer-2 output chunks
    NG = GRP * SEG_W    # max moving width of group-level MLP matmuls

    # group sizes: 4-window groups, but finish with 2/1/1 so the final MLP
    # chains are short and overlap the x DMA stream instead of trailing it
    gplan = []
    rem = windows
    while rem > 4:
        gplan.append(4)
        rem -= 4
    gplan.extend([2, 1, 1] if rem == 4 else [1] * rem)
    gstarts = [sum(gplan[:i]) for i in range(len(gplan))]
    group_of = {}
    for gi, (wg, gsz) in enumerate(zip(gstarts, gplan)):
        for w in range(wg, wg + gsz):
            group_of[w] = gi

    is_eq = mybir.AluOpType.is_equal
    DR = mybir.MatmulPerfMode.DoubleRow

    with tile.TileContext(nc) as tc, ExitStack() as ctx:
        cpool = ctx.enter_context(tc.tile_pool(name="consts", bufs=1))
        xpool = ctx.enter_context(tc.tile_pool(name="x", bufs=5))
        ohpool = ctx.enter_context(tc.tile_pool(name="oh", bufs=4))
        spool = ctx.enter_context(tc.tile_pool(name="work", bufs=2))
        pg = ctx.enter_context(tc.tile_pool(name="pg", bufs=2, space=bass.MemorySpace.PSUM))
        pt = ctx.enter_context(tc.tile_pool(name="pt", bufs=2, space=bass.MemorySpace.PSUM))
        pm = ctx.enter_context(tc.tile_pool(name="pm", bufs=3, space=bass.MemorySpace.PSUM))

        # ---- constants / weights into SBUF ----
        warm_sb = cpool.tile([128, 128], dt.bfloat16)
        nc.vector.memset(warm_sb[:], 1.0)
        iotr_sb = cpool.tile([128, 128], dt.bfloat16)
        nc.sync.dma_start(iotr_sb[:], iotr)
        iden_sb = cpool.tile([128, 128], dt.float32)
        w1_sb = cpool.tile([128, HC, H], m_dt)
        w2_sb = cpool.tile([128, HC, H2], m_dt)
        w3_sb = cpool.tile([128, KC], m_dt)
        b1_sb = cpool.tile([128, JC], dt.float32)
        b2_sb = cpool.tile([128, KC], dt.float32)
        b3_sb = cpool.tile([1, 1], dt.float32)

        def emit_weight_dmas():
            nc.sync.dma_start(w1_sb[:], w1t.rearrange("(c p) j -> p c j", p=128))
            nc.sync.dma_start(w2_sb[:], w2t.rearrange("(c p) j -> p c j", p=128))
            nc.sync.dma_start(w3_sb[:], w3c)
            nc.sync.dma_start(b1_sb[:], b1c)
            nc.sync.dma_start(b2_sb[:], b2c)
            nc.sync.dma_start(b3_sb[:], b3c)
        brt_sb = cpool.tile([128, windows * t_tiles], dt.bfloat16)
        nc.sync.dma_start(brt_sb[:], brt)
        gw_sb = cpool.tile([FD + 1, H], m_dt)
        w1bw_sb = cpool.tile([FD + 1, H], m_dt)
        domE_sb = cpool.tile([FD + 1, spc], m_dt)
        out_sb = cpool.tile([1, spc], dt.float32)

        # ---- PE warm-up: ~4.5us of dummy matmuls on a memset tile (no DMA
        # dependency, starts immediately) so HAM flips to K=8/8 before the
        # real stream starts.
        warm_t = pm.tile([128, H], dt.float32, tag="pmlp")
        for i in range(44):
            nc.tensor.matmul(
                warm_t[:, 0:128], warm_sb[:], warm_sb[:],
                start=(i == 0), stop=(i == 43))
        nc.sync.dma_start(iden_sb[:], iden)
        nc.sync.dma_start(gw_sb[:], gw)
        nc.sync.dma_start(w1bw_sb[:], w1bw)
        nc.sync.dma_start(domE_sb[:], domE)

        gstate = {}

        def emit_gamma(gi):
            """gammaT for group gi: [128 h, HC, span] bf16."""
            wg, span = gstarts[gi], gplan[gi] * SEG_W
            gam = spool.tile([128, HC, NG], m_dt, tag="gam")
            dom_s = domE_sb[:, wg * SEG_W: wg * SEG_W + span]
            for hc in range(HC):
                pgb = pm.tile([128, H], dt.float32, tag="pmlp")
                nc.tensor.matmul(
                    pgb[:, 0:span],
                    gw_sb[:, hc * 128:(hc + 1) * 128], dom_s,
                    start=True, stop=True)
                nc.scalar.copy(gam[:, hc, 0:span], pgb[:, 0:span])
            gstate[gi] = (gam, spool.tile(
                [128, HC, NG], m_dt, tag="gmodT", name="gmodT"))

        def emit_mlp(wg, span, gi):
            """MLP for group gi = windows [wg, wg+span/SEG_W)."""
            gmodT = gstate[gi][1]
            dom_s = domE_sb[:, wg * SEG_W: wg * SEG_W + span]
            # layer 1 (+ folded beta/beta_b via K=17 matmul)
            h1 = spool.tile([128, HC, NG], m_dt, tag="h1")
            for jc in range(JC):
                ph1 = pm.tile([128, NG], dt.float32, tag="pmlp")
                nc.tensor.matmul(
                    ph1[:, 0:span],
                    w1bw_sb[:, jc * 128:(jc + 1) * 128], dom_s,
                    start=True, stop=False, skip_group_check=True)
                for hc in range(HC):
                    nc.tensor.matmul(
                        ph1[:, 0:span],
                        w1_sb[:, hc, jc * 128:(jc + 1) * 128],
                        gmodT[:, hc, 0:span],
                        start=False, stop=(hc == HC - 1), skip_group_check=True)
                if SILU_COMPOSE:
                    z1 = spool.tile([128, NG], dt.float32, tag="z1")
                    nc.scalar.activation(
                        z1[:, 0:span], ph1[:, 0:span],
                        mybir.ActivationFunctionType.Identity,
                        bias=b1_sb[:, jc:jc + 1])
                    nc.scalar.activation(
                        h1[:, jc, 0:span], z1[:, 0:span],
                        mybir.ActivationFunctionType.Sigmoid)
                    nc.vector.tensor_mul(
                        h1[:, jc, 0:span], h1[:, jc, 0:span], z1[:, 0:span])
                else:
                    nc.scalar.activation(
                        h1[:, jc, 0:span], ph1[:, 0:span],
                        mybir.ActivationFunctionType.Silu,
                        bias=b1_sb[:, jc:jc + 1])
            # layer 2
            h2 = spool.tile([128, KC, NG], m_dt, tag="h2")
            for kc in range(KC):
                ph2 = pm.tile([128, NG], dt.float32, tag="pmlp")
                for hc in range(HC):
                    nc.tensor.matmul(
                        ph2[:, 0:span],
                        w2_sb[:, hc, kc * 128:(kc + 1) * 128],
                        h1[:, hc, 0:span],
                        start=(hc == 0), stop=(hc == HC - 1))
                if SILU_COMPOSE:
                    z2 = spool.tile([128, NG], dt.float32, tag="z2")
                    nc.scalar.activation(
                        z2[:, 0:span], ph2[:, 0:span],
                        mybir.ActivationFunctionType.Identity,
                        bias=b2_sb[:, kc:kc + 1])
                    nc.scalar.activation(
                        h2[:, kc, 0:span], z2[:, 0:span],
                        mybir.ActivationFunctionType.Sigmoid)
                    nc.vector.tensor_mul(
                        h2[:, kc, 0:span], h2[:, kc, 0:span], z2[:, 0:span])
                else:
                    nc.scalar.activation(
                        h2[:, kc, 0:span], ph2[:, 0:span],
                        mybir.ActivationFunctionType.Silu,
                        bias=b2_sb[:, kc:kc + 1])
            # output head
            po = pm.tile([1, NG], dt.float32, tag="pmlp")
            for kc in range(KC):
                nc.tensor.matmul(
                    po[:, 0:span], w3_sb[:, kc:kc + 1], h2[:, kc, 0:span],
                    start=(kc == 0), stop=(kc == KC - 1))
            nc.scalar.activation(
                out_sb[0:1, wg * SEG_W: wg * SEG_W + span], po[:, 0:span],
                mybir.ActivationFunctionType.Identity,
                bias=b3_sb[0:1, 0:1])
            nc.sync.dma_start(
                out[0:1, wg * SEG_W: wg * SEG_W + span],
                out_sb[0:1, wg * SEG_W: wg * SEG_W + span])

        g_sbs = {}

        def emit_transform(w):
            """Transpose g(w) and FiLM-multiply into its group's gmodT."""
            g_sb = g_sbs.pop(w)
            pt_t = pt.tile([128, H], dt.float32, name="pt_t")
            for hc in range(HC):
                nc.tensor.transpose(
                    pt_t[:, hc * 128:(hc + 1) * 128],
                    g_sb[:, hc * 128:(hc + 1) * 128],
                    iden_sb[:])
            gi = group_of[w]
            wi = w - gstarts[gi]
            gam, gmodT = gstate[gi]
            pt_v = pt_t[:].rearrange("p (c s) -> p c s", c=HC)
            gm_v = gmodT[:].rearrange("p c (g s) -> p c g s", g=GRP)
            ga_v = gam[:].rearrange("p c (g s) -> p c g s", g=GRP)
            nc.vector.tensor_mul(
                gm_v[:, :, wi, :], pt_v, ga_v[:, :, wi, :])

        mlp_done = set()

        def emit_mlp_group(grp_i):
            if grp_i in mlp_done:
                return
            mlp_done.add(grp_i)
            emit_mlp(gstarts[grp_i], gplan[grp_i] * SEG_W, grp_i)

        for w in range(windows):
            if w in gstarts:
                emit_gamma(group_of[w])

            # --- segment-sum for this window: accumulate [128 seg, H] ---
            pg_t = pg.tile([128, H], dt.float32)
            off = 0
            for blk in blocks:
                x_sb = xpool.tile([128, XT, H], x_dt)
                nc.sync.dma_start(
                    x_sb[:, 0:blk, :],
                    xp[w][:, off * H:(off + blk) * H]
                    .rearrange("p (c h) -> p c h", c=blk))
                # batched one-hot for the whole block: [128, blk, 128] fp8
                oh = ohpool.tile([128, XT, 128], x_dt)
                iotr_v = iotr_sb[:].rearrange("p (o s) -> p o s", o=1)
                brt_v = brt_sb[:, w * t_tiles + off: w * t_tiles + off + blk]
                brt_v = brt_v.rearrange("p (c o) -> p c o", o=1)
                in0, in1 = bass.broadcast_tensor_aps(iotr_v, brt_v)
                nc.vector.tensor_tensor(oh[:, 0:blk, :], in0, in1, is_eq)
                for gpair in range(blk // 2):
                    ti = off + 2 * gpair
                    nc.tensor.matmul(
                        pg_t[:],
                        oh[:, 2 * gpair:2 * gpair + 2, :],
                        x_sb[:, 2 * gpair:2 * gpair + 2, :],
                        start=(ti == 0), stop=(ti == t_tiles - 2),
                        perf_mode=DR)
                off += blk

            # evict g(w) right behind its stop-matmul (ScalarE overlaps the
            # next window's PE stream; the transpose runs a window later)
            g_sb = spool.tile([128, H], dt.float32, tag="g", name="g_sb")
            nc.scalar.copy(g_sb[:], pg_t[:])
            g_sbs[w] = g_sb

            if w == 0:
                emit_weight_dmas()
            if w >= 1:
                emit_transform(w - 1)
            if w >= 2:
                g2 = group_of[w - 2]
                if w - 2 == gstarts[g2] + gplan[g2] - 1:
                    emit_mlp_group(g2)

        emit_transform(windows - 1)
        for grp_i in range(len(gplan)):
            emit_mlp_group(grp_i)

    nc.compile()
    return nc


def _sigma_delta_fp8(x: np.ndarray, batch: np.ndarray, n_segs: int) -> np.ndarray:
    """fp8-e4m3 quantization of x with per-(segment, h) error feedback.

    Within each segment the quantization errors telescope, so segment sums
    of the returned array match the fp32 sums to ~1 quantum.
    """
    starts = np.searchsorted(batch, np.arange(n_segs + 1))
    lens = np.diff(starts)
    L = int(lens.max())
    xq = np.empty(x.shape, dtype=F8)
    order = np.argsort(-lens, kind="stable")  # longest first: shrinking actives
    sorted_lens = lens[order]
    sorted_starts = starts[order]
    carry = np.zeros((n_segs, x.shape[1]), np.float32)
    for k in range(L):
        n_act = int(np.searchsorted(-sorted_lens, -k, side="left"))
        if n_act == 0:
            break
        rows = sorted_starts[:n_act] + k
        v = x[rows] + carry[:n_act]
        q = v.astype(F8)
        carry[:n_act] = v - q.astype(np.float32)
        xq[rows] = q
    return xq


def _balance_windows(batch: np.ndarray, n_segs: int, n_windows: int):
    """Greedy LPT assignment of segments to windows (128 segments each),
    equalizing node counts.  Returns (win_of_seg, slot_of_seg, t_tiles)."""
    counts = np.bincount(batch, minlength=n_segs)
    order = np.argsort(-counts, kind="stable")
    loads = np.zeros(n_windows, np.int64)
    nseg = np.zeros(n_windows, np.int64)
    win = np.empty(n_segs, np.int64)
    INF = 1 << 40
    for s in order:
        eligible = np.where(nseg < SEG_W, loads, INF)
        w = int(np.argmin(eligible))
        win[s] = w
        loads[w] += counts[s]
        nseg[w] += 1
    assert (nseg == SEG_W).all()
    # slot of each segment within its window (stable by segment id)
    o = np.argsort(win, kind="stable")
    slot = np.empty(n_segs, np.int64)
    slot[o] = np.arange(n_segs) - np.repeat(
        np.arange(n_windows) * SEG_W, SEG_W)
    t_tiles = max(2, 2 * int(-(-loads.max() // 256)))
    return win, slot, t_tiles


def prepare_core_inputs(
    x, batch, domain_emb, gamma_w, gamma_b, beta_w, beta_b,
    w1, b1, w2, b2, w3, b3,
    spc: int, n_cores: int, plan=None,
):
    """Slice/pad/pack the full inputs into one in_map per core.

    Returns (in_maps, seg_pos) where seg_pos[seg] is the segment's position
    in the permuted, concatenated output."""
    n_segs = spc * n_cores
    windows = spc // SEG_W
    n_win_tot = windows * n_cores

    batch = np.ascontiguousarray(np.asarray(batch).astype(np.int64))
    x = np.asarray(x, dtype=np.float32)

    if plan is None:
        plan = _balance_windows(batch, n_segs, n_win_tot)
    win, slot, t_tiles = plan
    npw = SEG_W * t_tiles

    w1_f = np.asarray(w1, np.float32)
    bw_ext = np.concatenate([np.asarray(beta_w, np.float32).T,
                             np.asarray(beta_b, np.float32)[None]], axis=0)  # [17, H]
    w1bw = bw_ext @ w1_f.T                                                   # [17, H]

    shared = {
        "gw": np.ascontiguousarray(_f32_to_bf16(
            np.concatenate([np.asarray(gamma_w, np.float32).T,
                            np.asarray(gamma_b, np.float32)[None]], axis=0))),
        "w1bw": np.ascontiguousarray(_f32_to_bf16(w1bw)),
        "w1t": np.ascontiguousarray(_f32_to_bf16(w1_f.T)),
        "w2t": np.ascontiguousarray(_f32_to_bf16(np.asarray(w2, np.float32).T)),
        "w3c": np.ascontiguousarray(
            _f32_to_bf16(np.asarray(w3, np.float32).reshape(H2 // 128, 128).T)),
        "b1c": np.ascontiguousarray(np.asarray(b1, np.float32).reshape(H // 128, 128).T),
        "b2c": np.ascontiguousarray(np.asarray(b2, np.float32).reshape(H2 // 128, 128).T),
        "b3c": np.asarray(b3, np.float32).reshape(1, 1),
        "iden": np.eye(128, dtype=np.float32),
        "iotr": np.tile(np.arange(128, dtype=np.float32), (128, 1)).astype(BF16),
    }

    xq_u8 = _sigma_delta_fp8(x, batch, n_segs).view(np.uint8)

    # permuted node order: grouped by window (stable, so per-segment runs stay
    # contiguous), with per-node window/slot ids
    node_win = win[batch]
    node_slot = slot[batch].astype(np.float32)
    order = np.argsort(node_win, kind="stable")
    wstarts = np.searchsorted(node_win[order], np.arange(n_win_tot + 1))

    # segment position in the permuted output
    seg_pos = win * SEG_W + slot

    dom = np.asarray(domain_emb, np.float32)
    dom_ext = np.concatenate([dom.T, np.ones((1, n_segs), np.float32)], axis=0)
    domP = np.empty((FD + 1, n_segs), np.float32)
    domP[:, seg_pos] = dom_ext

    in_maps = []
    for core in range(n_cores):
        xp_c = np.zeros((windows, npw, H), dtype=np.uint8)
        brt_c = np.full((windows, npw), -1024.0, dtype=np.float32)
        for wl in range(windows):
            wg = core * windows + wl
            ns = order[wstarts[wg]:wstarts[wg + 1]]
            cnt = len(ns)
            if cnt > npw:
                raise ValueError(f"window overflow: {cnt} > {npw}")
            if cnt == 0:
                continue
            xp_c[wl, :cnt] = xq_u8[ns]
            brt_c[wl, :cnt] = node_slot[ns]
        # [windows, npw, H] -> [windows, 128, t_tiles*H]: node c*128+p at
        # partition p, free slot (c, h)
        xp_c = np.ascontiguousarray(
            xp_c.reshape(windows, t_tiles, 128, H)
            .transpose(0, 2, 1, 3)
            .reshape(windows, 128, t_tiles * H)).view(F8)
        # [windows, npw] -> [128, windows*t_tiles]: brt[p, w*t_tiles+ti]
        brt_c = np.ascontiguousarray(
            brt_c.reshape(windows, t_tiles, 128).transpose(2, 0, 1)
            .reshape(128, windows * t_tiles).astype(BF16))
        domE_c = np.ascontiguousarray(
            _f32_to_bf16(domP[:, core * spc:(core + 1) * spc]))
        in_maps.append({"xp": xp_c, "brt": brt_c, "domE": domE_c, **shared})
    return in_maps, seg_pos, t_tiles


_PROGRAM_CACHE: dict = {}

# Set by test harnesses: request an NTFF trace and stash the raw results.
TRACE = False
LAST_RESULT = None


def kernel(**inputs) -> np.ndarray:
    x = np.asarray(inputs["x"], dtype=np.float32)
    batch = np.ascontiguousarray(np.asarray(inputs["batch"]).astype(np.int64))
    assert x.shape == (N_NODES, H), x.shape

    spc = B_SEGS // N_CORES

    in_maps, seg_pos, t_tiles = prepare_core_inputs(
        x, batch,
        inputs["domain_emb"], inputs["gamma_w"], inputs["gamma_b"],
        inputs["beta_w"], inputs["beta_b"],
        inputs["w1"], inputs["b1"], inputs["w2"], inputs["b2"],
        inputs["w3"], inputs["b3"],
        spc, N_CORES,
    )

    key = (spc, t_tiles, N_CORES)
    if key not in _PROGRAM_CACHE:
        _PROGRAM_CACHE[key] = build_program(spc, t_tiles, N_CORES)
    nc = _PROGRAM_CACHE[key]

    res = bass_utils.run_bass_kernel_spmd(
        nc, in_maps, core_ids=list(range(N_CORES)), trace=TRACE)
    global LAST_RESULT
    LAST_RESULT = res
    out_perm = np.concatenate(
        [res.results[c]["out"].reshape(-1) for c in range(N_CORES)])
    return np.ascontiguousarray(out_perm[seg_pos].astype(np.float32))


# revision 15
# speedup vs baseline: 2.4808x; 1.0050x over previous
"""Trainium2 Bass kernel for nn_GemNetOutput (segment_reduce + FiLM + MLP head).

Reference computation (all fp32):
    g     = segment_sum(x, batch, num_segments=B)        # [B, H]
    gamma = domain_emb @ gamma_w.T + gamma_b             # [B, H]
    beta  = domain_emb @ beta_w.T  + beta_b              # [B, H]
    g     = gamma * g + beta
    h     = silu(g @ w1.T + b1)                          # [B, H]
    h     = silu(h @ w2.T + b2)                          # [B, H/2]
    out   = (h @ w3.T + b3).squeeze(-1)                  # [B]

Shapes: N=1e6 nodes, B=16384 graphs, H=512, FD=16.  `batch` is SORTED.

Strategy (8 NeuronCores, no collectives needed):
  - The 16384 segments are BIN-PACKED into 128 windows of exactly 128
    segments each, equalizing the node count per window (greedy LPT on the
    host).  Core c owns windows [c*16, (c+1)*16).  All cores run one
    identical static program; every window is padded to the same t_tiles
    node tiles (sentinel one-hot ids mask the padding) -- balancing makes
    that padding ~1.5% instead of ~5%.
  - x is converted to fp8-e4m3 on the host with SIGMA-DELTA (error-feedback)
    rounding along each segment: quantization errors telescope within a
    segment, so the device's segment sums match the fp32 sums to ~1 quantum
    instead of sqrt(n) quanta.  Halves HBM traffic vs bf16.
  - x is packed on the host into [window, 128, t_tiles*H] so each DMA block
    is a fully contiguous ~1 MB transfer (8 KB per partition line).
  - segment_sum on the PE with fp8 DoubleRow matmuls: each matmul consumes
    TWO 128-node tiles (contraction 256) against a [128, 2, 128] one-hot.
  - One-hot built on DVE with one batched tensor_tensor per DMA block
    (stride-0 broadcast APs) instead of one tensor_scalar per tile.
  - beta (incl. beta_b) is folded into the MLP-1 accumulation on the host:
    ph1 += (W1 @ beta_w_ext.T) @ dom_ext, one small K=17 matmul per j-chunk.
  - FiLM multiply + MLP run per GROUP of 4 windows in transposed
    [feature, seg] layout so MLP matmuls have N=512 moving operands.
"""

import sys
from contextlib import ExitStack

for _p in ("/opt/trn_rl_repo", "/opt/pypackages"):
    if _p not in sys.path:
        sys.path.append(_p)

import ml_dtypes
import numpy as np

import concourse.bass as bass
import concourse.tile as tile
from concourse import bacc, mybir
from concourse import bass_utils

dt = mybir.dt

# Problem constants (hardcoded per the contract).
N_NODES = 1_000_000
B_SEGS = 16_384
H = 512
H2 = 256
FD = 16
N_CORES = 8
SEG_W = 128          # segments per window (PSUM partition dim)
GRP = 4              # windows per MLP group (moving N = GRP*SEG_W = 512)
XT = 32              # max node subtiles (of 128 rows) per x DMA block

BF16 = ml_dtypes.bfloat16
F8 = ml_dtypes.float8_e4m3fn

# CoreSim has no Silu LUT; compose silu = z * sigmoid(z) when True (sim tests).
SILU_COMPOSE = False


def _f32_to_bf16(a: np.ndarray) -> np.ndarray:
    return np.ascontiguousarray(a, dtype=np.float32).astype(BF16)


def _blocks_of(t_tiles: int) -> tuple:
    """Split t_tiles into even-sized DMA blocks of at most XT subtiles."""
    blocks = [XT] * (t_tiles // XT)
    rem = t_tiles % XT
    if rem:
        blocks.append(rem)
    assert all(b % 2 == 0 for b in blocks)
    return tuple(blocks)


def build_program(spc: int, t_tiles: int, n_cores: int):
    """Build the per-core Bass/Tile program.

    spc: segments per core (multiple of 128)
    t_tiles: node tiles (of 128) per 128-segment window; even
    """
    assert t_tiles % 2 == 0
    windows = spc // SEG_W
    blocks = _blocks_of(t_tiles)
    x_dt = dt.float8e4
    m_dt = dt.bfloat16             # MLP matmul dtype

    nc = bacc.Bacc(
        "TRN2",
        target_bir_lowering=False,
        debug=False,
        enable_asserts=False,
        num_devices=n_cores,
    )

    xp = nc.dram_tensor("xp", [windows, 128, t_tiles * H], x_dt,
                        kind="ExternalInput").ap()
    brt = nc.dram_tensor("brt", [128, windows * t_tiles], dt.bfloat16,
                         kind="ExternalInput").ap()
    domE = nc.dram_tensor("domE", [FD + 1, spc], m_dt, kind="ExternalInput").ap()
    gw = nc.dram_tensor("gw", [FD + 1, H], m_dt, kind="ExternalInput").ap()
    w1bw = nc.dram_tensor("w1bw", [FD + 1, H], m_dt, kind="ExternalInput").ap()
    w1t = nc.dram_tensor("w1t", [H, H], m_dt, kind="ExternalInput").ap()
    w2t = nc.dram_tensor("w2t", [H, H2], m_dt, kind="ExternalInput").ap()
    w3c = nc.dram_tensor("w3c", [128, H2 // 128], m_dt, kind="ExternalInput").ap()
    b1c = nc.dram_tensor("b1c", [128, H // 128], dt.float32, kind="ExternalInput").ap()
    b2c = nc.dram_tensor("b2c", [128, H2 // 128], dt.float32, kind="ExternalInput").ap()
    b3c = nc.dram_tensor("b3c", [1, 1], dt.float32, kind="ExternalInput").ap()
    iden = nc.dram_tensor("iden", [128, 128], dt.float32, kind="ExternalInput").ap()
    iotr = nc.dram_tensor("iotr", [128, 128], dt.bfloat16, kind="ExternalInput").ap()
    out = nc.dram_tensor("out", [1, spc], dt.float32, kind="ExternalOutput").ap()

    HC = H // 128       # 4 h-chunks
    JC = H // 128       # 4 layer-1 output chunks
    KC = H2 // 128      # 2 layer-2 output chunks
    NG = GRP * SEG_W    # max moving width of group-level MLP matmuls

    # group sizes: 4-window groups, but finish with 2/1/1 so the final MLP
    # chains are short and overlap the x DMA stream instead of trailing it
    gplan = []
    rem = windows
    while rem > 4:
        gplan.append(4)
        rem -= 4
    gplan.extend([2, 1, 1] if rem == 4 else [1] * rem)
    gstarts = [sum(gplan[:i]) for i in range(len(gplan))]
    group_of = {}
    for gi, (wg, gsz) in enumerate(zip(gstarts, gplan)):
        for w in range(wg, wg + gsz):
            group_of[w] = gi

    is_eq = mybir.AluOpType.is_equal
    DR = mybir.MatmulPerfMode.DoubleRow

    with tile.TileContext(nc) as tc, ExitStack() as ctx:
        cpool = ctx.enter_context(tc.tile_pool(name="consts", bufs=1))
        xpool = ctx.enter_context(tc.tile_pool(name="x", bufs=6))
        ohpool = ctx.enter_context(tc.tile_pool(name="oh", bufs=4))
        spool = ctx.enter_context(tc.tile_pool(name="work", bufs=2))
        pg = ctx.enter_context(tc.tile_pool(name="pg", bufs=2, space=bass.MemorySpace.PSUM))
        pt = ctx.enter_context(tc.tile_pool(name="pt", bufs=2, space=bass.MemorySpace.PSUM))
        pm = ctx.enter_context(tc.tile_pool(name="pm", bufs=3, space=bass.MemorySpace.PSUM))

        # ---- constants / weights into SBUF ----
        warm_sb = cpool.tile([128, 128], dt.bfloat16)
        nc.vector.memset(warm_sb[:], 1.0)
        iotr_sb = cpool.tile([128, 128], dt.bfloat16)
        nc.sync.dma_start(iotr_sb[:], iotr)
        iden_sb = cpool.tile([128, 128], dt.float32)
        w1_sb = cpool.tile([128, HC, H], m_dt)
        w2_sb = cpool.tile([128, HC, H2], m_dt)
        w3_sb = cpool.tile([128, KC], m_dt)
        b1_sb = cpool.tile([128, JC], dt.float32)
        b2_sb = cpool.tile([128, KC], dt.float32)
        b3_sb = cpool.tile([1, 1], dt.float32)

        def emit_weight_dmas():
            nc.sync.dma_start(w1_sb[:], w1t.rearrange("(c p) j -> p c j", p=128))
            nc.sync.dma_start(w2_sb[:], w2t.rearrange("(c p) j -> p c j", p=128))
            nc.sync.dma_start(w3_sb[:], w3c)
            nc.sync.dma_start(b1_sb[:], b1c)
            nc.sync.dma_start(b2_sb[:], b2c)
            nc.sync.dma_start(b3_sb[:], b3c)
        brt_sb = cpool.tile([128, windows * t_tiles], dt.bfloat16)
        nc.sync.dma_start(brt_sb[:], brt)
        gw_sb = cpool.tile([FD + 1, H], m_dt)
        w1bw_sb = cpool.tile([FD + 1, H], m_dt)
        domE_sb = cpool.tile([FD + 1, spc], m_dt)
        out_sb = cpool.tile([1, spc], dt.float32)

        # ---- PE warm-up: ~4.5us of dummy matmuls on a memset tile (no DMA
        # dependency, starts immediately) so HAM flips to K=8/8 before the
        # real stream starts.
        warm_t = pm.tile([128, H], dt.float32, tag="pmlp")
        for i in range(44):
            nc.tensor.matmul(
                warm_t[:, 0:128], warm_sb[:], warm_sb[:],
                start=(i == 0), stop=(i == 43))
        nc.sync.dma_start(iden_sb[:], iden)
        nc.sync.dma_start(gw_sb[:], gw)
        nc.sync.dma_start(w1bw_sb[:], w1bw)
        nc.sync.dma_start(domE_sb[:], domE)

        gstate = {}

        def emit_gamma(gi):
            """gammaT for group gi: [128 h, HC, span] bf16."""
            wg, span = gstarts[gi], gplan[gi] * SEG_W
            gam = spool.tile([128, HC, NG], m_dt, tag="gam")
            dom_s = domE_sb[:, wg * SEG_W: wg * SEG_W + span]
            for hc in range(HC):
                pgb = pm.tile([128, H], dt.float32, tag="pmlp")
                nc.tensor.matmul(
                    pgb[:, 0:span],
                    gw_sb[:, hc * 128:(hc + 1) * 128], dom_s,
                    start=True, stop=True)
                nc.scalar.copy(gam[:, hc, 0:span], pgb[:, 0:span])
            gstate[gi] = (gam, spool.tile(
                [128, HC, NG], m_dt, tag="gmodT", name="gmodT"))

        mlp_pieces = []   # queued thunks, drained a few per window

        def queue_mlp(wg, span, gi):
            """Queue the MLP for group gi as independently-emittable pieces."""
            gmodT = gstate[gi][1]
            dom_s = domE_sb[:, wg * SEG_W: wg * SEG_W + span]
            st = {}

            def mk_l1(jc):
                def piece():
                    if "h1" not in st:
                        st["h1"] = spool.tile(
                            [128, HC, NG], m_dt, tag="h1", name="h1")
                    h1 = st["h1"]
                    ph1 = pm.tile([128, NG], dt.float32, tag="pmlp")
                    nc.tensor.matmul(
                        ph1[:, 0:span],
                        w1bw_sb[:, jc * 128:(jc + 1) * 128], dom_s,
                        start=True, stop=False, skip_group_check=True)
                    for hc in range(HC):
                        nc.tensor.matmul(
                            ph1[:, 0:span],
                            w1_sb[:, hc, jc * 128:(jc + 1) * 128],
                            gmodT[:, hc, 0:span],
                            start=False, stop=(hc == HC - 1),
                            skip_group_check=True)
                    if SILU_COMPOSE:
                        z1 = spool.tile([128, NG], dt.float32, tag="z1")
                        nc.scalar.activation(
                            z1[:, 0:span], ph1[:, 0:span],
                            mybir.ActivationFunctionType.Identity,
                            bias=b1_sb[:, jc:jc + 1])
                        nc.scalar.activation(
                            h1[:, jc, 0:span], z1[:, 0:span],
                            mybir.ActivationFunctionType.Sigmoid)
                        nc.vector.tensor_mul(
                            h1[:, jc, 0:span], h1[:, jc, 0:span], z1[:, 0:span])
                    else:
                        nc.scalar.activation(
                            h1[:, jc, 0:span], ph1[:, 0:span],
                            mybir.ActivationFunctionType.Silu,
                            bias=b1_sb[:, jc:jc + 1])
                return piece

            def mk_l2(kc):
                def piece():
                    if "h2" not in st:
                        st["h2"] = spool.tile(
                            [128, KC, NG], m_dt, tag="h2", name="h2")
                    h1, h2 = st["h1"], st["h2"]
                    ph2 = pm.tile([128, NG], dt.float32, tag="pmlp")
                    for hc in range(HC):
                        nc.tensor.matmul(
                            ph2[:, 0:span],
                            w2_sb[:, hc, kc * 128:(kc + 1) * 128],
                            h1[:, hc, 0:span],
                            start=(hc == 0), stop=(hc == HC - 1))
                    if SILU_COMPOSE:
                        z2 = spool.tile([128, NG], dt.float32, tag="z2")
                        nc.scalar.activation(
                            z2[:, 0:span], ph2[:, 0:span],
                            mybir.ActivationFunctionType.Identity,
                            bias=b2_sb[:, kc:kc + 1])
                        nc.scalar.activation(
                            h2[:, kc, 0:span], z2[:, 0:span],
                            mybir.ActivationFunctionType.Sigmoid)
                        nc.vector.tensor_mul(
                            h2[:, kc, 0:span], h2[:, kc, 0:span], z2[:, 0:span])
                    else:
                        nc.scalar.activation(
                            h2[:, kc, 0:span], ph2[:, 0:span],
                            mybir.ActivationFunctionType.Silu,
                            bias=b2_sb[:, kc:kc + 1])
                return piece

            def head():
                h2 = st["h2"]
                po = pm.tile([1, NG], dt.float32, tag="pmlp")
                for kc in range(KC):
                    nc.tensor.matmul(
                        po[:, 0:span], w3_sb[:, kc:kc + 1], h2[:, kc, 0:span],
                        start=(kc == 0), stop=(kc == KC - 1))
                nc.scalar.activation(
                    out_sb[0:1, wg * SEG_W: wg * SEG_W + span], po[:, 0:span],
                    mybir.ActivationFunctionType.Identity,
                    bias=b3_sb[0:1, 0:1])
                nc.sync.dma_start(
                    out[0:1, wg * SEG_W: wg * SEG_W + span],
                    out_sb[0:1, wg * SEG_W: wg * SEG_W + span])

            for jc in range(JC):
                mlp_pieces.append(mk_l1(jc))
            for kc in range(KC):
                mlp_pieces.append(mk_l2(kc))
            mlp_pieces.append(head)

        g_sbs = {}

        def emit_transform(w):
            """Transpose g(w) and FiLM-multiply into its group's gmodT."""
            g_sb = g_sbs.pop(w)
            pt_t = pt.tile([128, H], dt.float32, name="pt_t")
            for hc in range(HC):
                nc.tensor.transpose(
                    pt_t[:, hc * 128:(hc + 1) * 128],
                    g_sb[:, hc * 128:(hc + 1) * 128],
                    iden_sb[:])
            gi = group_of[w]
            wi = w - gstarts[gi]
            gam, gmodT = gstate[gi]
            pt_v = pt_t[:].rearrange("p (c s) -> p c s", c=HC)
            gm_v = gmodT[:].rearrange("p c (g s) -> p c g s", g=GRP)
            ga_v = gam[:].rearrange("p c (g s) -> p c g s", g=GRP)
            nc.vector.tensor_mul(
                gm_v[:, :, wi, :], pt_v, ga_v[:, :, wi, :])

        mlp_done = set()

        def emit_mlp_group(grp_i):
            if grp_i in mlp_done:
                return
            mlp_done.add(grp_i)
            queue_mlp(gstarts[grp_i], gplan[grp_i] * SEG_W, grp_i)

        for w in range(windows):
            if w in gstarts:
                emit_gamma(group_of[w])

            # --- segment-sum for this window: accumulate [128 seg, H] ---
            pg_t = pg.tile([128, H], dt.float32)
            off = 0
            for blk in blocks:
                x_sb = xpool.tile([128, XT, H], x_dt)
                nc.sync.dma_start(
                    x_sb[:, 0:blk, :],
                    xp[w][:, off * H:(off + blk) * H]
                    .rearrange("p (c h) -> p c h", c=blk))
                # batched one-hot for the whole block: [128, blk, 128] fp8
                oh = ohpool.tile([128, XT, 128], x_dt)
                iotr_v = iotr_sb[:].rearrange("p (o s) -> p o s", o=1)
                brt_v = brt_sb[:, w * t_tiles + off: w * t_tiles + off + blk]
                brt_v = brt_v.rearrange("p (c o) -> p c o", o=1)
                in0, in1 = bass.broadcast_tensor_aps(iotr_v, brt_v)
                nc.vector.tensor_tensor(oh[:, 0:blk, :], in0, in1, is_eq)
                for gpair in range(blk // 2):
                    ti = off + 2 * gpair
                    nc.tensor.matmul(
                        pg_t[:],
                        oh[:, 2 * gpair:2 * gpair + 2, :],
                        x_sb[:, 2 * gpair:2 * gpair + 2, :],
                        start=(ti == 0), stop=(ti == t_tiles - 2),
                        perf_mode=DR)
                off += blk

            # evict g(w) right behind its stop-matmul (ScalarE overlaps the
            # next window's PE stream; the transpose runs a window later)
            g_sb = spool.tile([128, H], dt.float32, tag="g", name="g_sb")
            nc.scalar.copy(g_sb[:], pg_t[:])
            g_sbs[w] = g_sb

            if w == 0:
                emit_weight_dmas()
            if w >= 1:
                emit_transform(w - 1)
                g1 = group_of[w - 1]
                if w - 1 == gstarts[g1] + gplan[g1] - 1:
                    emit_mlp_group(g1)
            for _ in range(3):
                if mlp_pieces:
                    mlp_pieces.pop(0)()

        emit_transform(windows - 1)
        for grp_i in range(len(gplan)):
            emit_mlp_group(grp_i)
        while mlp_pieces:
            mlp_pieces.pop(0)()

    nc.compile()
    return nc


def _sigma_delta_fp8(x: np.ndarray, batch: np.ndarray, n_segs: int) -> np.ndarray:
    """fp8-e4m3 quantization of x with per-(segment, h) error feedback.

    Within each segment the quantization errors telescope, so segment sums
    of the returned array match the fp32 sums to ~1 quantum.
    """
    starts = np.searchsorted(batch, np.arange(n_segs + 1))
    lens = np.diff(starts)
    L = int(lens.max())
    xq = np.empty(x.shape, dtype=F8)
    order = np.argsort(-lens, kind="stable")  # longest first: shrinking actives
    sorted_lens = lens[order]
    sorted_starts = starts[order]
    carry = np.zeros((n_segs, x.shape[1]), np.float32)
    for k in range(L):
        n_act = int(np.searchsorted(-sorted_lens, -k, side="left"))
        if n_act == 0:
            break
        rows = sorted_starts[:n_act] + k
        v = x[rows] + carry[:n_act]
        q = v.astype(F8)
        carry[:n_act] = v - q.astype(np.float32)
        xq[rows] = q
    return xq


def _balance_windows(batch: np.ndarray, n_segs: int, n_windows: int):
    """Greedy LPT assignment of segments to windows (128 segments each),
    equalizing node counts.  Returns (win_of_seg, slot_of_seg, t_tiles)."""
    counts = np.bincount(batch, minlength=n_segs)
    order = np.argsort(-counts, kind="stable")
    loads = np.zeros(n_windows, np.int64)
    nseg = np.zeros(n_windows, np.int64)
    win = np.empty(n_segs, np.int64)
    INF = 1 << 40
    for s in order:
        eligible = np.where(nseg < SEG_W, loads, INF)
        w = int(np.argmin(eligible))
        win[s] = w
        loads[w] += counts[s]
        nseg[w] += 1
    assert (nseg == SEG_W).all()
    # slot of each segment within its window (stable by segment id)
    o = np.argsort(win, kind="stable")
    slot = np.empty(n_segs, np.int64)
    slot[o] = np.arange(n_segs) - np.repeat(
        np.arange(n_windows) * SEG_W, SEG_W)
    t_tiles = max(2, 2 * int(-(-loads.max() // 256)))
    return win, slot, t_tiles


def prepare_core_inputs(
    x, batch, domain_emb, gamma_w, gamma_b, beta_w, beta_b,
    w1, b1, w2, b2, w3, b3,
    spc: int, n_cores: int, plan=None,
):
    """Slice/pad/pack the full inputs into one in_map per core.

    Returns (in_maps, seg_pos) where seg_pos[seg] is the segment's position
    in the permuted, concatenated output."""
    n_segs = spc * n_cores
    windows = spc // SEG_W
    n_win_tot = windows * n_cores

    batch = np.ascontiguousarray(np.asarray(batch).astype(np.int64))
    x = np.asarray(x, dtype=np.float32)

    if plan is None:
        plan = _balance_windows(batch, n_segs, n_win_tot)
    win, slot, t_tiles = plan
    npw = SEG_W * t_tiles

    w1_f = np.asarray(w1, np.float32)
    bw_ext = np.concatenate([np.asarray(beta_w, np.float32).T,
                             np.asarray(beta_b, np.float32)[None]], axis=0)  # [17, H]
    w1bw = bw_ext @ w1_f.T                                                   # [17, H]

    shared = {
        "gw": np.ascontiguousarray(_f32_to_bf16(
            np.concatenate([np.asarray(gamma_w, np.float32).T,
                            np.asarray(gamma_b, np.float32)[None]], axis=0))),
        "w1bw": np.ascontiguousarray(_f32_to_bf16(w1bw)),
        "w1t": np.ascontiguousarray(_f32_to_bf16(w1_f.T)),
        "w2t": np.ascontiguousarray(_f32_to_bf16(np.asarray(w2, np.float32).T)),
        "w3c": np.ascontiguousarray(
            _f32_to_bf16(np.asarray(w3, np.float32).reshape(H2 // 128, 128).T)),
        "b1c": np.ascontiguousarray(np.asarray(b1, np.float32).reshape(H // 128, 128).T),
        "b2c": np.ascontiguousarray(np.asarray(b2, np.float32).reshape(H2 // 128, 128).T),
        "b3c": np.asarray(b3, np.float32).reshape(1, 1),
        "iden": np.eye(128, dtype=np.float32),
        "iotr": np.tile(np.arange(128, dtype=np.float32), (128, 1)).astype(BF16),
    }

    xq_u8 = _sigma_delta_fp8(x, batch, n_segs).view(np.uint8)

    # permuted node order: grouped by window (stable, so per-segment runs stay
    # contiguous), with per-node window/slot ids
    node_win = win[batch]
    node_slot = slot[batch].astype(np.float32)
    order = np.argsort(node_win, kind="stable")
    wstarts = np.searchsorted(node_win[order], np.arange(n_win_tot + 1))

    # segment position in the permuted output
    seg_pos = win * SEG_W + slot

    dom = np.asarray(domain_emb, np.float32)
    dom_ext = np.concatenate([dom.T, np.ones((1, n_segs), np.float32)], axis=0)
    domP = np.empty((FD + 1, n_segs), np.float32)
    domP[:, seg_pos] = dom_ext

    in_maps = []
    for core in range(n_cores):
        xp_c = np.zeros((windows, npw, H), dtype=np.uint8)
        brt_c = np.full((windows, npw), -1024.0, dtype=np.float32)
        for wl in range(windows):
            wg = core * windows + wl
            ns = order[wstarts[wg]:wstarts[wg + 1]]
            cnt = len(ns)
            if cnt > npw:
                raise ValueError(f"window overflow: {cnt} > {npw}")
            if cnt == 0:
                continue
            xp_c[wl, :cnt] = xq_u8[ns]
            brt_c[wl, :cnt] = node_slot[ns]
        # [windows, npw, H] -> [windows, 128, t_tiles*H]: node c*128+p at
        # partition p, free slot (c, h)
        xp_c = np.ascontiguousarray(
            xp_c.reshape(windows, t_tiles, 128, H)
            .transpose(0, 2, 1, 3)
            .reshape(windows, 128, t_tiles * H)).view(F8)
        # [windows, npw] -> [128, windows*t_tiles]: brt[p, w*t_tiles+ti]
        brt_c = np.ascontiguousarray(
            brt_c.reshape(windows, t_tiles, 128).transpose(2, 0, 1)
            .reshape(128, windows * t_tiles).astype(BF16))
        domE_c = np.ascontiguousarray(
            _f32_to_bf16(domP[:, core * spc:(core + 1) * spc]))
        in_maps.append({"xp": xp_c, "brt": brt_c, "domE": domE_c, **shared})
    return in_maps, seg_pos, t_tiles


_PROGRAM_CACHE: dict = {}

# Set by test harnesses: request an NTFF trace and stash the raw results.
TRACE = False
LAST_RESULT = None


def kernel(**inputs) -> np.ndarray:
    x = np.asarray(inputs["x"], dtype=np.float32)
    batch = np.ascontiguousarray(np.asarray(inputs["batch"]).astype(np.int64))
    assert x.shape == (N_NODES, H), x.shape

    spc = B_SEGS // N_CORES

    in_maps, seg_pos, t_tiles = prepare_core_inputs(
        x, batch,
        inputs["domain_emb"], inputs["gamma_w"], inputs["gamma_b"],
        inputs["beta_w"], inputs["beta_b"],
        inputs["w1"], inputs["b1"], inputs["w2"], inputs["b2"],
        inputs["w3"], inputs["b3"],
        spc, N_CORES,
    )

    key = (spc, t_tiles, N_CORES)
    if key not in _PROGRAM_CACHE:
        _PROGRAM_CACHE[key] = build_program(spc, t_tiles, N_CORES)
    nc = _PROGRAM_CACHE[key]

    res = bass_utils.run_bass_kernel_spmd(
        nc, in_maps, core_ids=list(range(N_CORES)), trace=TRACE)
    global LAST_RESULT
    LAST_RESULT = res
    out_perm = np.concatenate(
        [res.results[c]["out"].reshape(-1) for c in range(N_CORES)])
    return np.ascontiguousarray(out_perm[seg_pos].astype(np.float32))
